# revision 1
# baseline (speedup 1.0000x reference)
"""Trainium2 Bass kernel for nn_LinearUpscaler (masked embedding-bag sum + bias).

reference:  g = W.T[ids]; g[ids == 0] = 0; out = g.sum(axis=2) + b

Design: data-parallel over batch across 8 cores (8 batch rows -> 1600 bags of
51 slots each: 50 items + 1 bias-row slot; ids==0 remapped to a zero row).

The gather engine is the GPSIMD dma_gather custom op (int16 indices, one
512B/256B row per index, written to partition i%128, column i//128).  Since
indices are signed int16 (max 32767) and V=100002, the fp16 table is split
into 4 vocab chunks; row 0 of each chunk is a zero row so padding slots can
gather harmlessly.  For each tile of 128 bags, each bag's slots are bucketed
by chunk; per (tile, chunk) all bags are padded to a common width W so the
dma_gather index list is fully valid (no negative indices) with a
compile-time num_idxs.  The list order is chosen so bag b's rows land in
partition b at consecutive columns; one strided vector-engine reduce per tile
sums items+chunks+padding (pads contribute zeros) in a single pass, and the
result is DMA'd out.  The program is specialized per call (widths depend on
the actual ids); no collectives are needed.
"""

import importlib.util
import os
import sys

if importlib.util.find_spec("concourse") is None:
    for _p in ("/opt/trn_rl_repo", "/root/.axon_site/_ro/trn_rl_repo"):
        if os.path.isdir(_p) and _p not in sys.path:
            sys.path.insert(0, _p)
            break

import numpy as np

N_CORES = 8
B, S, K = 64, 200, 50
V, E = 100000, 128
KE = K + 1            # items + bias slot
BIAS_V = V            # logical row V   = b
ZERO_V = V + 1        # logical row V+1 = 0
NV = V + 2            # logical vocab incl. bias+zero rows
P = 128
B_PER_CORE = B // N_CORES
BAGS = B_PER_CORE * S               # 1600 bags per core
N_TILES = -(-BAGS // P)             # 13
BAGS_PAD = N_TILES * P              # 1664

CHUNK_CAP = 32767                   # real rows per chunk (idx 1..32767)
N_CHUNKS = -(-NV // CHUNK_CAP)      # 4
# chunk 3 is mostly empty (1701 natural rows); fill its spare idx space with
# DUPLICATES of chunk-0..2 rows (chosen per call, by usefulness for shedding
# over-full bags) so bags can rebalance slots across chunks, smoothing the
# per-chunk counts that drive padding
N_NAT3 = NV - 3 * CHUNK_CAP         # 1701 natural chunk-3 rows
DUP_BASE = N_NAT3 + 1               # chunk-3 idx of first duplicate
N_DUP = CHUNK_CAP - DUP_BASE + 1    # 31066 duplicate slots
CHUNK_ROWS = [CHUNK_CAP + 1] * 3 + [DUP_BASE + N_DUP]


def _pick_dup_ids(all_bags):
    """Choose the N_DUP most useful rows to duplicate into chunk 3: score each
    id by its occurrences inside (bag, chunk) pairs that are over-full."""
    C = all_bags // CHUNK_CAP
    cnt = np.stack([(C == c).sum(axis=1) for c in range(N_CHUNKS)], axis=1)
    level = 13  # bags want per-chunk counts near ~12.75
    need = cnt > level                      # [bags, NC]
    useful = need[np.arange(len(all_bags))[:, None], C] & (all_bags < 3 * CHUNK_CAP)
    score = np.bincount(
        all_bags[useful].astype(np.int64), minlength=3 * CHUNK_CAP
    )
    dup_ids = np.sort(np.argsort(-score, kind="stable")[:N_DUP])
    dup_rank = np.full(3 * CHUNK_CAP, -1, np.int64)
    dup_rank[dup_ids] = np.arange(N_DUP)
    return dup_ids, dup_rank

TRACE = False       # test.py flips this to profile
LAST_RESULTS = {}   # test.py reads exec_time_ns etc. from here


def _build_tables(W, b, dup_ids):
    """f32 chunk tables, each [zero row; <=CHUNK_CAP vocab rows].

    Separate tensors (not slices of one): the gather ucode's row addressing
    breaks when AP-base-offset + idx exceeds 32767 rows."""
    wt = np.zeros((NV, E), np.float32)
    wt[:V] = W.T
    wt[BIAS_V] = b
    tabs = []
    for c in range(3):
        t = np.zeros((CHUNK_ROWS[c], E), np.float32)
        t[1:] = wt[CHUNK_CAP * c : CHUNK_CAP * (c + 1)]
        tabs.append(t)
    t3 = np.zeros((CHUNK_ROWS[3], E), np.float32)
    t3[1 : 1 + N_NAT3] = wt[3 * CHUNK_CAP :]
    t3[DUP_BASE:] = wt[dup_ids]
    tabs.append(t3)
    return tabs


def _plan_core(v_bags, dup_rank):
    """v_bags: [BAGS, K] logical rows. Returns per-bag chunk-sorted idx lists
    and per-chunk counts, after rebalancing duplicate-eligible slots from
    over-full chunks 0..2 into chunk 3 to minimize each bag's max count.

    sorted_idx[bag, j] = local int16 idx of the bag's j-th slot when slots are
    ordered by (rebalanced) chunk; cnt[bag, c] = slots in chunk c."""
    C = v_bags // CHUNK_CAP                      # [BAGS, K] natural chunk
    elig = (v_bags < 3 * CHUNK_CAP) & (
        dup_rank[np.minimum(v_bags, 3 * CHUNK_CAP - 1)] >= 0
    )
    C2 = C.copy()
    for b in range(v_bags.shape[0]):
        row = C2[b]
        c = [(row == x).sum() for x in range(N_CHUNKS)]
        movable = [list(np.where((row == x) & elig[b])[0]) for x in range(3)]
        while True:
            moved = False
            for mx in sorted(range(3), key=lambda x: -c[x]):
                if c[mx] <= c[3] + 1:
                    break
                if movable[mx]:
                    row[movable[mx].pop()] = 3
                    c[mx] -= 1
                    c[3] += 1
                    moved = True
                    break
            if not moved:
                break
    IDX = np.where(
        C2 == C,
        v_bags - C * CHUNK_CAP + 1,
        DUP_BASE + dup_rank[np.minimum(v_bags, 3 * CHUNK_CAP - 1)],
    ).astype(np.int16)
    order = np.argsort(C2, axis=1, kind="stable")  # chunk-major slot order
    IDX_sorted = np.take_along_axis(IDX, order, axis=1)
    cnt = np.stack([(C2 == c).sum(axis=1) for c in range(N_CHUNKS)], axis=1)
    return IDX_sorted, cnt


def _cluster_once(cnt, order):
    m = np.zeros((N_TILES, N_CHUNKS), np.int64)
    fill = np.zeros(N_TILES, np.int64)
    tiles = np.full((N_TILES, P), -1, np.int64)
    for b in order:
        best_key, best_t = None, None
        for t in range(N_TILES):
            if fill[t] >= P:
                continue
            inc = int(np.maximum(m[t], cnt[b]).sum() - m[t].sum())
            key = (inc, -int(fill[t]))
            if best_key is None or key < best_key:
                best_key, best_t = key, t
        tiles[best_t, fill[best_t]] = b
        m[best_t] = np.maximum(m[best_t], cnt[b])
        fill[best_t] += 1
    return tiles, m


def _refine(tiles, cnt_ext, iters=120):
    """Swap-based local search: repeatedly swap a bag out of the widest tile
    when it lowers the summed per-tile per-chunk maxima."""

    def tile_m(t):
        return cnt_ext[tiles[t]].max(axis=0)

    def max_without(members):
        """[P, NC] per-chunk max over members excluding each member."""
        ct = cnt_ext[members]
        srt = np.sort(ct, axis=0)
        top1, top2 = srt[-1], srt[-2]
        is_top = ct == top1[None, :]
        uniq = is_top.sum(axis=0) == 1
        return ct, np.where(is_top & uniq[None, :], top2[None, :], top1[None, :])

    m = np.stack([tile_m(t) for t in range(N_TILES)])
    for _ in range(iters):
        t = int(m.sum(axis=1).argmax())
        ct, m_wo_t = max_without(tiles[t])
        others = [u for u in range(N_TILES) if u != t]
        cb_list, m_wo_list = zip(*(max_without(tiles[u]) for u in others))
        cb = np.concatenate(cb_list)           # [M, NC] candidate counts
        m_wo_u = np.concatenate(m_wo_list)     # [M, NC] u's width w/o candidate
        # widths of t after swapping member i with candidate j
        new_t = np.maximum(m_wo_t[:, None, :], cb[None, :, :])  # [P, M, NC]
        d_t = new_t.sum(axis=2) - m[t].sum()
        # exact widths of u after losing candidate j and receiving member i
        new_u = np.maximum(m_wo_u[None, :, :], ct[:, None, :])  # [P, M, NC]
        u_sums = np.repeat(m[others].sum(axis=1), P)
        d_u = new_u.sum(axis=2) - u_sums[None, :]
        delta = d_t + d_u
        i, j = np.unravel_index(int(delta.argmin()), delta.shape)
        if delta[i, j] >= 0:
            break
        u_idx = others[j // P]  # j indexes (tile-in-others, slot)
        slot = j % P
        tiles[t][i], tiles[u_idx][slot] = tiles[u_idx][slot], tiles[t][i]
        m[t] = tile_m(t)
        m[u_idx] = tile_m(u_idx)
    return tiles, m


def _cluster(cnt):
    """Greedy-pack 1600 bags into 13 tiles of 128 minimizing sum of per-tile
    per-chunk maxima; best of a few orderings. Returns tiles [N_TILES, P] of
    bag ids (-1 = dummy)."""
    orders = [
        np.argsort(-cnt.max(axis=1), kind="stable"),
        np.argsort(-cnt[:, :3].max(axis=1), kind="stable"),
        np.lexsort((cnt[:, 2], cnt[:, 1], cnt[:, 0]))[::-1],
    ]
    rng = np.random.default_rng(0)
    base = np.argsort(-cnt.max(axis=1), kind="stable")
    for _ in range(12):
        # perturbed difficulty order: keeps hard bags early but varies packing
        noise = rng.normal(0, 1.5, size=len(cnt))
        orders.append(np.argsort(-(cnt.max(axis=1) + noise), kind="stable"))
    for _ in range(3):
        orders.append(rng.permutation(len(cnt)))
    best = None
    for order in orders:
        tiles, m = _cluster_once(cnt, order)
        tot = int(m.sum())
        if best is None or tot < best[0]:
            best = (tot, tiles, m)
    _, tiles, m = best
    # -1 dummies index the appended all-zeros row of cnt_ext
    cnt_ext = np.vstack([cnt, np.zeros((1, N_CHUNKS), cnt.dtype)])
    tiles, m = _refine(tiles, cnt_ext)
    # sort tiles by descending total width so tiles align across cores
    tw = m.sum(axis=1)
    order_t = np.argsort(-tw, kind="stable")
    return tiles[order_t], m[order_t]


def _wrap_idxs(arr, w):
    """arr [P, w] int16 (partition-major slot grid) -> [128, w*8] wrapped+replicated."""
    L = P * w
    i = np.arange(L)
    lin = arr[i % P, i // P]                     # list position i = col*128 + p
    wrapped = lin.reshape(w * 8, 16).T           # [16, w*8]
    return np.tile(wrapped, (8, 1)).astype(np.int16)


def _prep_inputs(content_input, W, b):
    """Returns (in_maps, widths) where widths[t][c] is shared across cores."""
    ids = np.asarray(content_input).astype(np.int64).reshape(B, S, K)
    Wf = np.asarray(W, dtype=np.float32)
    bf = np.asarray(b, dtype=np.float32)

    ids = np.where(ids == 0, ZERO_V, ids)
    dup_ids, dup_rank = _pick_dup_ids(ids.reshape(B * S, K))
    tabs = _build_tables(Wf, bf, dup_ids)
    per_core = []
    tiles_per_core = []
    widths = np.zeros((N_TILES, N_CHUNKS), np.int64)
    for c in range(N_CORES):
        bag_ids = ids[c * B_PER_CORE : (c + 1) * B_PER_CORE].reshape(BAGS, K)
        plan = _plan_core(bag_ids, dup_rank)  # bias added on-device
        per_core.append(plan)
        tiles, m = _cluster(plan[1])
        tiles_per_core.append(tiles)
        widths = np.maximum(widths, m)

    bias_rep = np.ascontiguousarray(np.tile(bf[None, :], (P, 1)).astype(np.float32))
    in_maps = []
    for core in range(N_CORES):
        IDX_sorted, cnt = per_core[core]
        tiles = tiles_per_core[core]
        planes = []
        for t in range(N_TILES):
            bags = tiles[t]  # [P] bag ids, -1 = dummy
            real = bags >= 0
            bsafe = np.where(real, bags, 0)
            csum = np.zeros((P,), np.int64)
            for ch in range(N_CHUNKS):
                w = int(widths[t][ch])
                if w == 0:
                    continue
                cn = np.where(real, cnt[bsafe, ch], 0)
                j = np.arange(w)[None, :]
                src = np.take_along_axis(
                    IDX_sorted[bsafe], np.minimum(csum[:, None] + j, K - 1), axis=1
                )
                arr = np.where((j < cn[:, None]) & real[:, None], src, 0).astype(
                    np.int16
                )
                planes.append(_wrap_idxs(arr, w))
                csum += cn
        ids16 = np.concatenate(planes, axis=1)
        m = {"ids16": np.ascontiguousarray(ids16), "bias": bias_rep}
        for c in range(N_CHUNKS):
            m[f"wt{c}"] = tabs[c]
        in_maps.append(m)
    return in_maps, widths, tiles_per_core


def _build_program(widths, ids_cols):
    import concourse.bass as bass
    import concourse.mybir as mybir
    from concourse import bacc
    from concourse.tile import TileContext

    sw = [int(widths[t].sum()) for t in range(N_TILES)]
    sw_max = max(sw)
    # g tile is sw_max*512B per partition; keep total pool under ~160KB/partition
    g_bufs = max(1, min(3, (160 * 1024) // (sw_max * E * 4)))

    nc = bacc.Bacc("TRN2", target_bir_lowering=False, debug=False, num_devices=N_CORES)
    ids_d = nc.declare_dram_parameter("ids16", [P, ids_cols], mybir.dt.int16, isOutput=False)
    wt_ds = [
        nc.declare_dram_parameter(
            f"wt{c}", [CHUNK_ROWS[c], E], mybir.dt.float32, isOutput=False
        )
        for c in range(N_CHUNKS)
    ]
    bias_d = nc.declare_dram_parameter("bias", [P, E], mybir.dt.float32, isOutput=False)
    out_d = nc.declare_dram_parameter("out", [BAGS_PAD, E], mybir.dt.float32, isOutput=True)

    with TileContext(nc) as tc:
        with (
            tc.tile_pool(name="ids", bufs=1) as ids_pool,
            tc.tile_pool(name="bias", bufs=1) as bias_pool,
            tc.tile_pool(name="g", bufs=g_bufs) as g_pool,
            tc.tile_pool(name="o", bufs=3) as o_pool,
        ):
            bias_sb = bias_pool.tile([P, E], mybir.dt.float32)
            nc.sync.dma_start(out=bias_sb[:], in_=bias_d[:])
            ids_sb = ids_pool.tile([P, ids_cols], mybir.dt.int16)
            nc.sync.dma_start(out=ids_sb[:], in_=ids_d[:])
            off = 0  # free-dim offset into ids16, in idx elements
            for t in range(N_TILES):
                g = g_pool.tile([P, sw_max * E], mybir.dt.float32)
                col = 0
                for ch in range(N_CHUNKS):
                    w = int(widths[t][ch])
                    if w == 0:
                        continue
                    n = w * P
                    dst = g[:, col * E : (col + w) * E].rearrange(
                        "p (j e) -> p j e", j=w, e=E
                    )
                    nc.gpsimd.dma_gather(
                        dst,
                        wt_ds[ch][:],
                        ids_sb[:, off : off + w * 8],
                        n,
                        n,
                        E,
                        single_packet=False,
                    )
                    col += w
                    off += w * 8
                o = o_pool.tile([P, E], mybir.dt.float32)
                nc.vector.tensor_reduce(
                    out=o[:],
                    in_=g[:, : sw[t] * E].rearrange("p (j e) -> p e j", j=sw[t], e=E),
                    axis=mybir.AxisListType.X,
                    op=mybir.AluOpType.add,
                )
                nc.vector.tensor_add(out=o[:], in0=o[:], in1=bias_sb[:])
                nc.sync.dma_start(out=out_d[t * P : (t + 1) * P, :], in_=o[:])
    nc.compile()
    return nc


def kernel(content_input, W, b):
    global LAST_RESULTS
    in_maps, widths, tiles_per_core = _prep_inputs(content_input, W, b)
    ids_cols = in_maps[0]["ids16"].shape[1]
    nc = _build_program(widths, ids_cols)

    from concourse.bass_utils import run_bass_kernel_spmd

    res = run_bass_kernel_spmd(nc, in_maps, list(range(N_CORES)), trace=TRACE)
    LAST_RESULTS = {
        "exec_time_ns": res.exec_time_ns,
        "mean_exec_time_ns": res.mean_exec_time_ns,
        "instructions_and_trace": res.instructions_and_trace,
        "profile_json": res.profile_json,
        "widths": widths,
    }

    out = np.empty((B, S, E), np.float32)
    for c in range(N_CORES):
        rows = res.results[c]["out"]  # [BAGS_PAD, E], permuted bag order
        flat = np.empty((BAGS, E), np.float32)
        tiles = tiles_per_core[c].reshape(BAGS_PAD)
        real = tiles >= 0
        flat[tiles[real]] = rows[real]
        out[c * B_PER_CORE : (c + 1) * B_PER_CORE] = flat.reshape(B_PER_CORE, S, E)
    return out



# revision 8
# speedup vs baseline: 2.1789x; 2.1789x over previous
"""Trainium2 Bass kernel for nn_LinearUpscaler (masked embedding-bag sum + bias).

reference:  g = W.T[ids]; g[ids == 0] = 0; out = g.sum(axis=2) + b

Design: data-parallel over batch across 8 cores (8 batch rows -> 1600 bags of
51 slots each: 50 items + 1 bias-row slot; ids==0 remapped to a zero row).

The gather engine is the GPSIMD dma_gather custom op (int16 indices, one
512B/256B row per index, written to partition i%128, column i//128).  Since
indices are signed int16 (max 32767) and V=100002, the fp16 table is split
into 4 vocab chunks; row 0 of each chunk is a zero row so padding slots can
gather harmlessly.  For each tile of 128 bags, each bag's slots are bucketed
by chunk; per (tile, chunk) all bags are padded to a common width W so the
dma_gather index list is fully valid (no negative indices) with a
compile-time num_idxs.  The list order is chosen so bag b's rows land in
partition b at consecutive columns; one strided vector-engine reduce per tile
sums items+chunks+padding (pads contribute zeros) in a single pass, and the
result is DMA'd out.  The program is specialized per call (widths depend on
the actual ids); no collectives are needed.
"""

import importlib.util
import os
import sys

if importlib.util.find_spec("concourse") is None:
    for _p in ("/opt/trn_rl_repo", "/root/.axon_site/_ro/trn_rl_repo"):
        if os.path.isdir(_p) and _p not in sys.path:
            sys.path.insert(0, _p)
            break

import numpy as np

try:
    import ml_dtypes
    _BF16 = ml_dtypes.bfloat16
except ImportError:
    _BF16 = None


def _to_bf16(a):
    """f32 -> bf16 (round-to-nearest-even); uint16 view fallback."""
    if _BF16 is not None:
        return a.astype(_BF16)
    x = np.ascontiguousarray(a, np.float32).view(np.uint32)
    r = ((x >> 16) & 1) + 0x7FFF
    return ((x + r) >> 16).astype(np.uint16)


N_CORES = 8
B, S, K = 64, 200, 50
V, E = 100000, 128
KE = K + 1            # items + bias slot
BIAS_V = V            # logical row V   = b
ZERO_V = V + 1        # logical row V+1 = 0
NV = V + 2            # logical vocab incl. bias+zero rows
P = 128
B_PER_CORE = B // N_CORES
BAGS = B_PER_CORE * S               # 1600 bags per core
N_TILES = -(-BAGS // P)             # 13
BAGS_PAD = N_TILES * P              # 1664

CHUNK_CAP = 32767                   # real rows per chunk (idx 1..32767)
N_CHUNKS = -(-NV // CHUNK_CAP)      # 4
# chunk 3 is mostly empty (1701 natural rows); fill its spare idx space with
# DUPLICATES of chunk-0..2 rows (chosen per call, by usefulness for shedding
# over-full bags) so bags can rebalance slots across chunks, smoothing the
# per-chunk counts that drive padding
N_NAT3 = NV - 3 * CHUNK_CAP         # 1701 natural chunk-3 rows
DUP_BASE = N_NAT3 + 1               # chunk-3 idx of first duplicate
N_DUP = CHUNK_CAP - DUP_BASE + 1    # 31066 duplicate slots
CHUNK_ROWS = [CHUNK_CAP + 1] * 3 + [DUP_BASE + N_DUP]


def _pick_dup_ids(all_bags):
    """Choose the N_DUP most useful rows to duplicate into chunk 3: score each
    id by its occurrences inside (bag, chunk) pairs that are over-full."""
    C = all_bags // CHUNK_CAP
    cnt = np.stack([(C == c).sum(axis=1) for c in range(N_CHUNKS)], axis=1)
    level = 13  # bags want per-chunk counts near ~12.75
    need = cnt > level                      # [bags, NC]
    useful = need[np.arange(len(all_bags))[:, None], C] & (all_bags < 3 * CHUNK_CAP)
    score = np.bincount(
        all_bags[useful].astype(np.int64), minlength=3 * CHUNK_CAP
    )
    dup_ids = np.sort(np.argsort(-score, kind="stable")[:N_DUP])
    dup_rank = np.full(3 * CHUNK_CAP, -1, np.int64)
    dup_rank[dup_ids] = np.arange(N_DUP)
    return dup_ids, dup_rank

TRACE = False       # test.py flips this to profile
LAST_RESULTS = {}   # test.py reads exec_time_ns etc. from here


def _build_tables(W, b, dup_ids):
    """bf16 chunk tables, each [zero row; <=CHUNK_CAP vocab rows].

    Separate tensors (not slices of one): the gather ucode's row addressing
    breaks when AP-base-offset + idx exceeds 32767 rows."""
    wt = np.zeros((NV, E), np.float32)
    wt[:V] = W.T
    wt[BIAS_V] = b
    tabs = []
    for c in range(3):
        t = np.zeros((CHUNK_ROWS[c], E), np.float32)
        t[1:] = wt[CHUNK_CAP * c : CHUNK_CAP * (c + 1)]
        tabs.append(_to_bf16(t))
    t3 = np.zeros((CHUNK_ROWS[3], E), np.float32)
    t3[1 : 1 + N_NAT3] = wt[3 * CHUNK_CAP :]
    t3[DUP_BASE:] = wt[dup_ids]
    tabs.append(_to_bf16(t3))
    return tabs


def _plan_core(v_bags, dup_rank):
    """v_bags: [BAGS, K] logical rows. Returns per-bag chunk-sorted idx lists
    and per-chunk counts, after rebalancing duplicate-eligible slots from
    over-full chunks 0..2 into chunk 3 to minimize each bag's max count.

    sorted_idx[bag, j] = local int16 idx of the bag's j-th slot when slots are
    ordered by (rebalanced) chunk; cnt[bag, c] = slots in chunk c."""
    C = v_bags // CHUNK_CAP                      # [BAGS, K] natural chunk
    elig = (v_bags < 3 * CHUNK_CAP) & (
        dup_rank[np.minimum(v_bags, 3 * CHUNK_CAP - 1)] >= 0
    )
    C2 = C.copy()
    for b in range(v_bags.shape[0]):
        row = C2[b]
        c = [(row == x).sum() for x in range(N_CHUNKS)]
        movable = [list(np.where((row == x) & elig[b])[0]) for x in range(3)]
        while True:
            moved = False
            for mx in sorted(range(3), key=lambda x: -c[x]):
                if c[mx] <= c[3] + 1:
                    break
                if movable[mx]:
                    row[movable[mx].pop()] = 3
                    c[mx] -= 1
                    c[3] += 1
                    moved = True
                    break
            if not moved:
                break
    IDX = np.where(
        C2 == C,
        v_bags - C * CHUNK_CAP + 1,
        DUP_BASE + dup_rank[np.minimum(v_bags, 3 * CHUNK_CAP - 1)],
    ).astype(np.int16)
    order = np.argsort(C2, axis=1, kind="stable")  # chunk-major slot order
    IDX_sorted = np.take_along_axis(IDX, order, axis=1)
    cnt = np.stack([(C2 == c).sum(axis=1) for c in range(N_CHUNKS)], axis=1)
    return IDX_sorted, cnt


def _cluster_once(cnt, order):
    m = np.zeros((N_TILES, N_CHUNKS), np.int64)
    fill = np.zeros(N_TILES, np.int64)
    tiles = np.full((N_TILES, P), -1, np.int64)
    for b in order:
        best_key, best_t = None, None
        for t in range(N_TILES):
            if fill[t] >= P:
                continue
            inc = int(np.maximum(m[t], cnt[b]).sum() - m[t].sum())
            key = (inc, -int(fill[t]))
            if best_key is None or key < best_key:
                best_key, best_t = key, t
        tiles[best_t, fill[best_t]] = b
        m[best_t] = np.maximum(m[best_t], cnt[b])
        fill[best_t] += 1
    return tiles, m


def _refine(tiles, cnt_ext, iters=120):
    """Swap-based local search: repeatedly swap a bag out of the widest tile
    when it lowers the summed per-tile per-chunk maxima."""

    def tile_m(t):
        return cnt_ext[tiles[t]].max(axis=0)

    def max_without(members):
        """[P, NC] per-chunk max over members excluding each member."""
        ct = cnt_ext[members]
        srt = np.sort(ct, axis=0)
        top1, top2 = srt[-1], srt[-2]
        is_top = ct == top1[None, :]
        uniq = is_top.sum(axis=0) == 1
        return ct, np.where(is_top & uniq[None, :], top2[None, :], top1[None, :])

    m = np.stack([tile_m(t) for t in range(N_TILES)])
    for _ in range(iters):
        t = int(m.sum(axis=1).argmax())
        ct, m_wo_t = max_without(tiles[t])
        others = [u for u in range(N_TILES) if u != t]
        cb_list, m_wo_list = zip(*(max_without(tiles[u]) for u in others))
        cb = np.concatenate(cb_list)           # [M, NC] candidate counts
        m_wo_u = np.concatenate(m_wo_list)     # [M, NC] u's width w/o candidate
        # widths of t after swapping member i with candidate j
        new_t = np.maximum(m_wo_t[:, None, :], cb[None, :, :])  # [P, M, NC]
        d_t = new_t.sum(axis=2) - m[t].sum()
        # exact widths of u after losing candidate j and receiving member i
        new_u = np.maximum(m_wo_u[None, :, :], ct[:, None, :])  # [P, M, NC]
        u_sums = np.repeat(m[others].sum(axis=1), P)
        d_u = new_u.sum(axis=2) - u_sums[None, :]
        delta = d_t + d_u
        i, j = np.unravel_index(int(delta.argmin()), delta.shape)
        if delta[i, j] >= 0:
            break
        u_idx = others[j // P]  # j indexes (tile-in-others, slot)
        slot = j % P
        tiles[t][i], tiles[u_idx][slot] = tiles[u_idx][slot], tiles[t][i]
        m[t] = tile_m(t)
        m[u_idx] = tile_m(u_idx)
    return tiles, m


def _cluster(cnt):
    """Greedy-pack 1600 bags into 13 tiles of 128 minimizing sum of per-tile
    per-chunk maxima; best of a few orderings. Returns tiles [N_TILES, P] of
    bag ids (-1 = dummy)."""
    orders = [
        np.argsort(-cnt.max(axis=1), kind="stable"),
        np.argsort(-cnt[:, :3].max(axis=1), kind="stable"),
        np.lexsort((cnt[:, 2], cnt[:, 1], cnt[:, 0]))[::-1],
    ]
    rng = np.random.default_rng(0)
    base = np.argsort(-cnt.max(axis=1), kind="stable")
    for _ in range(12):
        # perturbed difficulty order: keeps hard bags early but varies packing
        noise = rng.normal(0, 1.5, size=len(cnt))
        orders.append(np.argsort(-(cnt.max(axis=1) + noise), kind="stable"))
    for _ in range(3):
        orders.append(rng.permutation(len(cnt)))
    best = None
    for order in orders:
        tiles, m = _cluster_once(cnt, order)
        tot = int(m.sum())
        if best is None or tot < best[0]:
            best = (tot, tiles, m)
    _, tiles, m = best
    # -1 dummies index the appended all-zeros row of cnt_ext
    cnt_ext = np.vstack([cnt, np.zeros((1, N_CHUNKS), cnt.dtype)])
    tiles, m = _refine(tiles, cnt_ext)
    # sort tiles by descending total width so tiles align across cores
    tw = m.sum(axis=1)
    order_t = np.argsort(-tw, kind="stable")
    return tiles[order_t], m[order_t]


def _wrap_idxs(arr, w):
    """arr [P, w] int16 (partition-major slot grid) -> [128, w*8] wrapped+replicated."""
    L = P * w
    i = np.arange(L)
    lin = arr[i % P, i // P]                     # list position i = col*128 + p
    wrapped = lin.reshape(w * 8, 16).T           # [16, w*8]
    return np.tile(wrapped, (8, 1)).astype(np.int16)


def _prep_inputs(content_input, W, b):
    """Returns (in_maps, widths) where widths[t][c] is shared across cores."""
    ids = np.asarray(content_input).astype(np.int64).reshape(B, S, K)
    Wf = np.asarray(W, dtype=np.float32)
    bf = np.asarray(b, dtype=np.float32)

    ids = np.where(ids == 0, ZERO_V, ids)
    dup_ids, dup_rank = _pick_dup_ids(ids.reshape(B * S, K))
    tabs = _build_tables(Wf, bf, dup_ids)
    per_core = []
    tiles_per_core = []
    widths = np.zeros((N_TILES, N_CHUNKS), np.int64)
    for c in range(N_CORES):
        bag_ids = ids[c * B_PER_CORE : (c + 1) * B_PER_CORE].reshape(BAGS, K)
        plan = _plan_core(bag_ids, dup_rank)  # bias added on-device
        per_core.append(plan)
        tiles, m = _cluster(plan[1])
        tiles_per_core.append(tiles)
        widths = np.maximum(widths, m)

    bias_rep = np.ascontiguousarray(np.tile(bf[None, :], (P, 1)).astype(np.float32))
    in_maps = []
    for core in range(N_CORES):
        IDX_sorted, cnt = per_core[core]
        tiles = tiles_per_core[core]
        planes = []
        for t in range(N_TILES):
            bags = tiles[t]  # [P] bag ids, -1 = dummy
            real = bags >= 0
            bsafe = np.where(real, bags, 0)
            csum = np.zeros((P,), np.int64)
            for ch in range(N_CHUNKS):
                w = int(widths[t][ch])
                if w == 0:
                    continue
                cn = np.where(real, cnt[bsafe, ch], 0)
                j = np.arange(w)[None, :]
                src = np.take_along_axis(
                    IDX_sorted[bsafe], np.minimum(csum[:, None] + j, K - 1), axis=1
                )
                arr = np.where((j < cn[:, None]) & real[:, None], src, 0).astype(
                    np.int16
                )
                planes.append(_wrap_idxs(arr, w))
                csum += cn
        ids16 = np.concatenate(planes, axis=1)
        m = {"ids16": np.ascontiguousarray(ids16), "bias": bias_rep}
        for c in range(N_CHUNKS):
            m[f"wt{c}"] = tabs[c]
        in_maps.append(m)
    return in_maps, widths, tiles_per_core


def _build_program(widths, ids_cols):
    import concourse.bass as bass
    import concourse.mybir as mybir
    from concourse import bacc
    from concourse.tile import TileContext

    sw = [int(widths[t].sum()) for t in range(N_TILES)]
    sw_max = max(sw)
    # g tile is sw_max*256B per partition; keep total pool under ~160KB/partition
    g_bufs = max(1, min(4, (160 * 1024) // (sw_max * E * 2)))

    nc = bacc.Bacc("TRN2", target_bir_lowering=False, debug=False,
                   num_devices=N_CORES, num_swdge_queues=4)
    ids_d = nc.declare_dram_parameter("ids16", [P, ids_cols], mybir.dt.int16, isOutput=False)
    wt_ds = [
        nc.declare_dram_parameter(
            f"wt{c}", [CHUNK_ROWS[c], E], mybir.dt.bfloat16, isOutput=False
        )
        for c in range(N_CHUNKS)
    ]
    bias_d = nc.declare_dram_parameter("bias", [P, E], mybir.dt.float32, isOutput=False)
    out_d = nc.declare_dram_parameter("out", [BAGS_PAD, E], mybir.dt.float32, isOutput=True)

    with TileContext(nc) as tc:
        with (
            tc.tile_pool(name="ids", bufs=1) as ids_pool,
            tc.tile_pool(name="bias", bufs=1) as bias_pool,
            tc.tile_pool(name="g", bufs=g_bufs) as g_pool,
            tc.tile_pool(name="o", bufs=3) as o_pool,
        ):
            bias_sb = bias_pool.tile([P, E], mybir.dt.float32)
            nc.sync.dma_start(out=bias_sb[:], in_=bias_d[:])
            ids_sb = ids_pool.tile([P, ids_cols], mybir.dt.int16)
            nc.sync.dma_start(out=ids_sb[:], in_=ids_d[:])
            off = 0  # free-dim offset into ids16, in idx elements
            qi = 0   # emitted-gather counter; queue = qi % 4 keeps each of
            # Tile's 8 round-robin DMASW lanes locked to a single SWDGE queue
            for t in range(N_TILES):
                g = g_pool.tile([P, sw_max * E], mybir.dt.bfloat16)
                col = 0
                for ch in range(N_CHUNKS):
                    w = int(widths[t][ch])
                    if w == 0:
                        continue
                    n = w * P
                    dst = g[:, col * E : (col + w) * E].rearrange(
                        "p (j e) -> p j e", j=w, e=E
                    )
                    nc.gpsimd.dma_gather(
                        dst,
                        wt_ds[ch][:],
                        ids_sb[:, off : off + w * 8],
                        n,
                        n,
                        E,
                        single_packet=False,
                        queue_num=qi % 4,
                    )
                    qi += 1
                    col += w
                    off += w * 8
                o = o_pool.tile([P, E], mybir.dt.float32)
                nc.vector.tensor_reduce(
                    out=o[:],
                    in_=g[:, : sw[t] * E].rearrange("p (j e) -> p e j", j=sw[t], e=E),
                    axis=mybir.AxisListType.X,
                    op=mybir.AluOpType.add,
                )
                nc.vector.tensor_add(out=o[:], in0=o[:], in1=bias_sb[:])
                nc.sync.dma_start(out=out_d[t * P : (t + 1) * P, :], in_=o[:])
    nc.compile()
    return nc


def kernel(content_input, W, b):
    global LAST_RESULTS
    in_maps, widths, tiles_per_core = _prep_inputs(content_input, W, b)
    ids_cols = in_maps[0]["ids16"].shape[1]
    nc = _build_program(widths, ids_cols)

    from concourse.bass_utils import run_bass_kernel_spmd

    res = run_bass_kernel_spmd(nc, in_maps, list(range(N_CORES)), trace=TRACE)
    LAST_RESULTS = {
        "exec_time_ns": res.exec_time_ns,
        "mean_exec_time_ns": res.mean_exec_time_ns,
        "instructions_and_trace": res.instructions_and_trace,
        "profile_json": res.profile_json,
        "widths": widths,
    }

    out = np.empty((B, S, E), np.float32)
    for c in range(N_CORES):
        rows = res.results[c]["out"]  # [BAGS_PAD, E], permuted bag order
        flat = np.empty((BAGS, E), np.float32)
        tiles = tiles_per_core[c].reshape(BAGS_PAD)
        real = tiles >= 0
        flat[tiles[real]] = rows[real]
        out[c * B_PER_CORE : (c + 1) * B_PER_CORE] = flat.reshape(B_PER_CORE, S, E)
    return out



# revision 11
# speedup vs baseline: 2.3903x; 1.0970x over previous
"""Trainium2 Bass kernel for nn_LinearUpscaler (masked embedding-bag sum + bias).

reference:  g = W.T[ids]; g[ids == 0] = 0; out = g.sum(axis=2) + b

Design: data-parallel over batch across 8 cores (8 batch rows -> 1600 bags of
51 slots each: 50 items + 1 bias-row slot; ids==0 remapped to a zero row).

The gather engine is the GPSIMD dma_gather custom op (int16 indices, one
512B/256B row per index, written to partition i%128, column i//128).  Since
indices are signed int16 (max 32767) and V=100002, the fp16 table is split
into 4 vocab chunks; row 0 of each chunk is a zero row so padding slots can
gather harmlessly.  For each tile of 128 bags, each bag's slots are bucketed
by chunk; per (tile, chunk) all bags are padded to a common width W so the
dma_gather index list is fully valid (no negative indices) with a
compile-time num_idxs.  The list order is chosen so bag b's rows land in
partition b at consecutive columns; one strided vector-engine reduce per tile
sums items+chunks+padding (pads contribute zeros) in a single pass, and the
result is DMA'd out.  The program is specialized per call (widths depend on
the actual ids); no collectives are needed.
"""

import importlib.util
import os
import sys

if importlib.util.find_spec("concourse") is None:
    for _p in ("/opt/trn_rl_repo", "/root/.axon_site/_ro/trn_rl_repo"):
        if os.path.isdir(_p) and _p not in sys.path:
            sys.path.insert(0, _p)
            break

import numpy as np

try:
    import ml_dtypes
    _BF16 = ml_dtypes.bfloat16
except ImportError:
    _BF16 = None


def _to_bf16(a):
    """f32 -> bf16 (round-to-nearest-even); uint16 view fallback."""
    if _BF16 is not None:
        return a.astype(_BF16)
    x = np.ascontiguousarray(a, np.float32).view(np.uint32)
    r = ((x >> 16) & 1) + 0x7FFF
    return ((x + r) >> 16).astype(np.uint16)


N_CORES = 8
B, S, K = 64, 200, 50
V, E = 100000, 128
KE = K + 1            # items + bias slot
BIAS_V = V            # logical row V   = b
ZERO_V = V + 1        # logical row V+1 = 0
NV = V + 2            # logical vocab incl. bias+zero rows
P = 128
B_PER_CORE = B // N_CORES
BAGS = B_PER_CORE * S               # 1600 bags per core
N_TILES = -(-BAGS // P)             # 13
BAGS_PAD = N_TILES * P              # 1664

CHUNK_CAP = 32767                   # real rows per chunk (idx 1..32767)
N_CHUNKS = -(-NV // CHUNK_CAP)      # 4
# chunk 3 is mostly empty (1701 natural rows); fill its spare idx space with
# DUPLICATES of chunk-0..2 rows (chosen per call, by usefulness for shedding
# over-full bags) so bags can rebalance slots across chunks, smoothing the
# per-chunk counts that drive padding
N_NAT3 = NV - 3 * CHUNK_CAP         # 1701 natural chunk-3 rows
DUP_BASE = N_NAT3 + 1               # chunk-3 idx of first duplicate
N_DUP = CHUNK_CAP - DUP_BASE + 1    # 31066 duplicate slots
CHUNK_ROWS = [CHUNK_CAP + 1] * 3 + [DUP_BASE + N_DUP]


def _pick_dup_ids(all_bags):
    """Choose the N_DUP most useful rows to duplicate into chunk 3: score each
    id by its occurrences inside (bag, chunk) pairs that are over-full."""
    C = all_bags // CHUNK_CAP
    cnt = np.stack([(C == c).sum(axis=1) for c in range(N_CHUNKS)], axis=1)
    level = 13  # bags want per-chunk counts near ~12.75
    need = cnt > level                      # [bags, NC]
    useful = need[np.arange(len(all_bags))[:, None], C] & (all_bags < 3 * CHUNK_CAP)
    score = np.bincount(
        all_bags[useful].astype(np.int64), minlength=3 * CHUNK_CAP
    )
    dup_ids = np.sort(np.argsort(-score, kind="stable")[:N_DUP])
    dup_rank = np.full(3 * CHUNK_CAP, -1, np.int64)
    dup_rank[dup_ids] = np.arange(N_DUP)
    return dup_ids, dup_rank

TRACE = False       # test.py flips this to profile
LAST_RESULTS = {}   # test.py reads exec_time_ns etc. from here


def _build_tables(W, b, dup_ids):
    """bf16 chunk tables, each [zero row; <=CHUNK_CAP vocab rows].

    Separate tensors (not slices of one): the gather ucode's row addressing
    breaks when AP-base-offset + idx exceeds 32767 rows."""
    wt = np.zeros((NV, E), np.float32)
    wt[:V] = W.T
    wt[BIAS_V] = b
    tabs = []
    for c in range(3):
        t = np.zeros((CHUNK_ROWS[c], E), np.float32)
        t[1:] = wt[CHUNK_CAP * c : CHUNK_CAP * (c + 1)]
        tabs.append(_to_bf16(t))
    t3 = np.zeros((CHUNK_ROWS[3], E), np.float32)
    t3[1 : 1 + N_NAT3] = wt[3 * CHUNK_CAP :]
    t3[DUP_BASE:] = wt[dup_ids]
    tabs.append(_to_bf16(t3))
    return tabs


def _plan_core(v_bags, dup_rank):
    """v_bags: [BAGS, K] logical rows. Returns per-bag chunk-sorted idx lists
    and per-chunk counts, after rebalancing duplicate-eligible slots from
    over-full chunks 0..2 into chunk 3 to minimize each bag's max count.

    sorted_idx[bag, j] = local int16 idx of the bag's j-th slot when slots are
    ordered by (rebalanced) chunk; cnt[bag, c] = slots in chunk c."""
    C = v_bags // CHUNK_CAP                      # [BAGS, K] natural chunk
    elig = (v_bags < 3 * CHUNK_CAP) & (
        dup_rank[np.minimum(v_bags, 3 * CHUNK_CAP - 1)] >= 0
    )
    C2 = C.copy()
    for b in range(v_bags.shape[0]):
        row = C2[b]
        c = [(row == x).sum() for x in range(N_CHUNKS)]
        movable = [list(np.where((row == x) & elig[b])[0]) for x in range(3)]
        while True:
            moved = False
            for mx in sorted(range(3), key=lambda x: -c[x]):
                if c[mx] <= c[3] + 1:
                    break
                if movable[mx]:
                    row[movable[mx].pop()] = 3
                    c[mx] -= 1
                    c[3] += 1
                    moved = True
                    break
            if not moved:
                break
    IDX = np.where(
        C2 == C,
        v_bags - C * CHUNK_CAP + 1,
        DUP_BASE + dup_rank[np.minimum(v_bags, 3 * CHUNK_CAP - 1)],
    ).astype(np.int16)
    order = np.argsort(C2, axis=1, kind="stable")  # chunk-major slot order
    IDX_sorted = np.take_along_axis(IDX, order, axis=1)
    cnt = np.stack([(C2 == c).sum(axis=1) for c in range(N_CHUNKS)], axis=1)
    return IDX_sorted, cnt


def _cluster_once(cnt, order):
    m = np.zeros((N_TILES, N_CHUNKS), np.int64)
    fill = np.zeros(N_TILES, np.int64)
    tiles = np.full((N_TILES, P), -1, np.int64)
    for b in order:
        best_key, best_t = None, None
        for t in range(N_TILES):
            if fill[t] >= P:
                continue
            inc = int(np.maximum(m[t], cnt[b]).sum() - m[t].sum())
            key = (inc, -int(fill[t]))
            if best_key is None or key < best_key:
                best_key, best_t = key, t
        tiles[best_t, fill[best_t]] = b
        m[best_t] = np.maximum(m[best_t], cnt[b])
        fill[best_t] += 1
    return tiles, m


def _refine(tiles, cnt_ext, iters=120):
    """Swap-based local search: repeatedly swap a bag out of the widest tile
    when it lowers the summed per-tile per-chunk maxima."""

    def tile_m(t):
        return cnt_ext[tiles[t]].max(axis=0)

    def max_without(members):
        """[P, NC] per-chunk max over members excluding each member."""
        ct = cnt_ext[members]
        srt = np.sort(ct, axis=0)
        top1, top2 = srt[-1], srt[-2]
        is_top = ct == top1[None, :]
        uniq = is_top.sum(axis=0) == 1
        return ct, np.where(is_top & uniq[None, :], top2[None, :], top1[None, :])

    m = np.stack([tile_m(t) for t in range(N_TILES)])
    for _ in range(iters):
        t = int(m.sum(axis=1).argmax())
        ct, m_wo_t = max_without(tiles[t])
        others = [u for u in range(N_TILES) if u != t]
        cb_list, m_wo_list = zip(*(max_without(tiles[u]) for u in others))
        cb = np.concatenate(cb_list)           # [M, NC] candidate counts
        m_wo_u = np.concatenate(m_wo_list)     # [M, NC] u's width w/o candidate
        # widths of t after swapping member i with candidate j
        new_t = np.maximum(m_wo_t[:, None, :], cb[None, :, :])  # [P, M, NC]
        d_t = new_t.sum(axis=2) - m[t].sum()
        # exact widths of u after losing candidate j and receiving member i
        new_u = np.maximum(m_wo_u[None, :, :], ct[:, None, :])  # [P, M, NC]
        u_sums = np.repeat(m[others].sum(axis=1), P)
        d_u = new_u.sum(axis=2) - u_sums[None, :]
        delta = d_t + d_u
        i, j = np.unravel_index(int(delta.argmin()), delta.shape)
        if delta[i, j] >= 0:
            break
        u_idx = others[j // P]  # j indexes (tile-in-others, slot)
        slot = j % P
        tiles[t][i], tiles[u_idx][slot] = tiles[u_idx][slot], tiles[t][i]
        m[t] = tile_m(t)
        m[u_idx] = tile_m(u_idx)
    return tiles, m


def _refine_balance(tiles, cnt_ext, iters=400):
    """Second local-search phase: minimize (max per-tile total width, sum),
    so the 13 tiles pipeline evenly (no heavy head-of-line tile)."""

    def tile_m(t):
        return cnt_ext[tiles[t]].max(axis=0)

    def max_without(members):
        ct = cnt_ext[members]
        srt = np.sort(ct, axis=0)
        top1, top2 = srt[-1], srt[-2]
        is_top = ct == top1[None, :]
        uniq = is_top.sum(axis=0) == 1
        return ct, np.where(is_top & uniq[None, :], top2[None, :], top1[None, :])

    m = np.stack([tile_m(t) for t in range(N_TILES)])
    for _ in range(iters):
        sw = m.sum(axis=1)
        t = int(sw.argmax())
        ct, m_wo_t = max_without(tiles[t])
        others = [u for u in range(N_TILES) if u != t]
        cb_list, m_wo_list = zip(*(max_without(tiles[u]) for u in others))
        cb = np.concatenate(cb_list)
        m_wo_u = np.concatenate(m_wo_list)
        new_t_sw = np.maximum(m_wo_t[:, None, :], cb[None, :, :]).sum(axis=2)
        new_u_sw = np.maximum(m_wo_u[None, :, :], ct[:, None, :]).sum(axis=2)
        u_sw = np.repeat(sw[others], P)
        # sum-of-squares: equalizes tile widths without inflating the total
        score = (new_t_sw**2 + new_u_sw**2) - (sw[t] ** 2 + u_sw[None, :] ** 2)
        i, j = np.unravel_index(int(score.argmin()), score.shape)
        if score[i, j] >= 0:
            break
        u_idx = others[j // P]
        slot = j % P
        tiles[t][i], tiles[u_idx][slot] = tiles[u_idx][slot], tiles[t][i]
        m[t] = tile_m(t)
        m[u_idx] = tile_m(u_idx)
    return tiles, m


def _cluster(cnt):
    """Greedy-pack 1600 bags into 13 tiles of 128 minimizing sum of per-tile
    per-chunk maxima; best of a few orderings. Returns tiles [N_TILES, P] of
    bag ids (-1 = dummy)."""
    orders = [
        np.argsort(-cnt.max(axis=1), kind="stable"),
        np.argsort(-cnt[:, :3].max(axis=1), kind="stable"),
        np.lexsort((cnt[:, 2], cnt[:, 1], cnt[:, 0]))[::-1],
    ]
    rng = np.random.default_rng(0)
    base = np.argsort(-cnt.max(axis=1), kind="stable")
    for _ in range(12):
        # perturbed difficulty order: keeps hard bags early but varies packing
        noise = rng.normal(0, 1.5, size=len(cnt))
        orders.append(np.argsort(-(cnt.max(axis=1) + noise), kind="stable"))
    for _ in range(3):
        orders.append(rng.permutation(len(cnt)))
    best = None
    for order in orders:
        tiles, m = _cluster_once(cnt, order)
        tot = int(m.sum())
        if best is None or tot < best[0]:
            best = (tot, tiles, m)
    _, tiles, m = best
    # -1 dummies index the appended all-zeros row of cnt_ext
    cnt_ext = np.vstack([cnt, np.zeros((1, N_CHUNKS), cnt.dtype)])
    tiles, m = _refine(tiles, cnt_ext)
    tiles, m = _refine_balance(tiles, cnt_ext)
    # lightest tile first: fills the gather/reduce pipeline fastest
    tw = m.sum(axis=1)
    order_t = np.argsort(tw, kind="stable")
    return tiles[order_t], m[order_t]


def _wrap_idxs(arr, w):
    """arr [P, w] int16 (partition-major slot grid) -> [128, w*8] wrapped+replicated."""
    L = P * w
    i = np.arange(L)
    lin = arr[i % P, i // P]                     # list position i = col*128 + p
    wrapped = lin.reshape(w * 8, 16).T           # [16, w*8]
    return np.tile(wrapped, (8, 1)).astype(np.int16)


def _prep_inputs(content_input, W, b):
    """Returns (in_maps, widths) where widths[t][c] is shared across cores."""
    ids = np.asarray(content_input).astype(np.int64).reshape(B, S, K)
    Wf = np.asarray(W, dtype=np.float32)
    bf = np.asarray(b, dtype=np.float32)

    ids = np.where(ids == 0, ZERO_V, ids)
    dup_ids, dup_rank = _pick_dup_ids(ids.reshape(B * S, K))
    tabs = _build_tables(Wf, bf, dup_ids)
    per_core = []
    tiles_per_core = []
    widths = np.zeros((N_TILES, N_CHUNKS), np.int64)
    for c in range(N_CORES):
        bag_ids = ids[c * B_PER_CORE : (c + 1) * B_PER_CORE].reshape(BAGS, K)
        plan = _plan_core(bag_ids, dup_rank)  # bias added on-device
        per_core.append(plan)
        tiles, m = _cluster(plan[1])
        tiles_per_core.append(tiles)
        widths = np.maximum(widths, m)

    bias_rep = np.ascontiguousarray(np.tile(bf[None, :], (P, 1)).astype(np.float32))
    in_maps = []
    for core in range(N_CORES):
        IDX_sorted, cnt = per_core[core]
        tiles = tiles_per_core[core]
        planes = []
        for t in range(N_TILES):
            bags = tiles[t]  # [P] bag ids, -1 = dummy
            real = bags >= 0
            bsafe = np.where(real, bags, 0)
            csum = np.zeros((P,), np.int64)
            for ch in range(N_CHUNKS):
                w = int(widths[t][ch])
                if w == 0:
                    continue
                cn = np.where(real, cnt[bsafe, ch], 0)
                j = np.arange(w)[None, :]
                src = np.take_along_axis(
                    IDX_sorted[bsafe], np.minimum(csum[:, None] + j, K - 1), axis=1
                )
                arr = np.where((j < cn[:, None]) & real[:, None], src, 0).astype(
                    np.int16
                )
                planes.append(_wrap_idxs(arr, w))
                csum += cn
        ids16 = np.concatenate(planes, axis=1)
        m = {"ids16": np.ascontiguousarray(ids16), "bias": bias_rep}
        for c in range(N_CHUNKS):
            m[f"wt{c}"] = tabs[c]
        in_maps.append(m)
    return in_maps, widths, tiles_per_core


def _build_program(widths, ids_cols):
    import concourse.bass as bass
    import concourse.mybir as mybir
    from concourse import bacc
    from concourse.tile import TileContext

    sw = [int(widths[t].sum()) for t in range(N_TILES)]
    sw_max = max(sw)
    # g tile is sw_max*256B per partition; keep total pool under ~160KB/partition
    g_bufs = max(1, min(6, (160 * 1024) // (sw_max * E * 2)))

    nc = bacc.Bacc("TRN2", target_bir_lowering=False, debug=False,
                   num_devices=N_CORES, num_swdge_queues=4)
    ids_d = nc.declare_dram_parameter("ids16", [P, ids_cols], mybir.dt.int16, isOutput=False)
    wt_ds = [
        nc.declare_dram_parameter(
            f"wt{c}", [CHUNK_ROWS[c], E], mybir.dt.bfloat16, isOutput=False
        )
        for c in range(N_CHUNKS)
    ]
    bias_d = nc.declare_dram_parameter("bias", [P, E], mybir.dt.float32, isOutput=False)
    out_d = nc.declare_dram_parameter("out", [BAGS_PAD, E], mybir.dt.float32, isOutput=True)

    with TileContext(nc) as tc:
        with (
            tc.tile_pool(name="ids", bufs=1) as ids_pool,
            tc.tile_pool(name="bias", bufs=1) as bias_pool,
            tc.tile_pool(name="g", bufs=g_bufs) as g_pool,
            tc.tile_pool(name="o", bufs=3) as o_pool,
        ):
            bias_sb = bias_pool.tile([P, E], mybir.dt.float32)
            nc.sync.dma_start(out=bias_sb[:], in_=bias_d[:])
            ids_sb = ids_pool.tile([P, ids_cols], mybir.dt.int16)
            nc.sync.dma_start(out=ids_sb[:], in_=ids_d[:])
            off = 0  # free-dim offset into ids16, in idx elements
            qi = 0   # emitted-gather counter; queue = qi % 4 keeps each of
            # Tile's 8 round-robin DMASW lanes locked to a single SWDGE queue
            for t in range(N_TILES):
                g = g_pool.tile([P, sw_max * E], mybir.dt.bfloat16)
                col = 0
                for ch in range(N_CHUNKS):
                    w = int(widths[t][ch])
                    if w == 0:
                        continue
                    n = w * P
                    dst = g[:, col * E : (col + w) * E].rearrange(
                        "p (j e) -> p j e", j=w, e=E
                    )
                    nc.gpsimd.dma_gather(
                        dst,
                        wt_ds[ch][:],
                        ids_sb[:, off : off + w * 8],
                        n,
                        n,
                        E,
                        single_packet=False,
                        queue_num=qi % 4,
                    )
                    qi += 1
                    col += w
                    off += w * 8
                o = o_pool.tile([P, E], mybir.dt.float32)
                nc.vector.tensor_reduce(
                    out=o[:],
                    in_=g[:, : sw[t] * E].rearrange("p (j e) -> p e j", j=sw[t], e=E),
                    axis=mybir.AxisListType.X,
                    op=mybir.AluOpType.add,
                )
                nc.vector.tensor_add(out=o[:], in0=o[:], in1=bias_sb[:])
                nc.sync.dma_start(out=out_d[t * P : (t + 1) * P, :], in_=o[:])
    nc.compile()
    return nc


def kernel(content_input, W, b):
    global LAST_RESULTS
    in_maps, widths, tiles_per_core = _prep_inputs(content_input, W, b)
    ids_cols = in_maps[0]["ids16"].shape[1]
    nc = _build_program(widths, ids_cols)

    from concourse.bass_utils import run_bass_kernel_spmd

    res = run_bass_kernel_spmd(nc, in_maps, list(range(N_CORES)), trace=TRACE)
    LAST_RESULTS = {
        "exec_time_ns": res.exec_time_ns,
        "mean_exec_time_ns": res.mean_exec_time_ns,
        "instructions_and_trace": res.instructions_and_trace,
        "profile_json": res.profile_json,
        "widths": widths,
    }

    out = np.empty((B, S, E), np.float32)
    for c in range(N_CORES):
        rows = res.results[c]["out"]  # [BAGS_PAD, E], permuted bag order
        flat = np.empty((BAGS, E), np.float32)
        tiles = tiles_per_core[c].reshape(BAGS_PAD)
        real = tiles >= 0
        flat[tiles[real]] = rows[real]
        out[c * B_PER_CORE : (c + 1) * B_PER_CORE] = flat.reshape(B_PER_CORE, S, E)
    return out



# revision 15
# speedup vs baseline: 2.6814x; 1.1218x over previous
"""Trainium2 Bass kernel for nn_LinearUpscaler (masked embedding-bag sum + bias).

reference:  g = W.T[ids]; g[ids == 0] = 0; out = g.sum(axis=2) + b

Design: data-parallel over batch across 8 cores (8 batch rows -> 1600 bags of
51 slots each: 50 items + 1 bias-row slot; ids==0 remapped to a zero row).

The gather engine is the GPSIMD dma_gather custom op (int16 indices, one
512B/256B row per index, written to partition i%128, column i//128).  Since
indices are signed int16 (max 32767) and V=100002, the fp16 table is split
into 4 vocab chunks; row 0 of each chunk is a zero row so padding slots can
gather harmlessly.  For each tile of 128 bags, each bag's slots are bucketed
by chunk; per (tile, chunk) all bags are padded to a common width W so the
dma_gather index list is fully valid (no negative indices) with a
compile-time num_idxs.  The list order is chosen so bag b's rows land in
partition b at consecutive columns; one strided vector-engine reduce per tile
sums items+chunks+padding (pads contribute zeros) in a single pass, and the
result is DMA'd out.  The program is specialized per call (widths depend on
the actual ids); no collectives are needed.
"""

import importlib.util
import os
import sys

if importlib.util.find_spec("concourse") is None:
    for _p in ("/opt/trn_rl_repo", "/root/.axon_site/_ro/trn_rl_repo"):
        if os.path.isdir(_p) and _p not in sys.path:
            sys.path.insert(0, _p)
            break

import numpy as np

try:
    import ml_dtypes
    _BF16 = ml_dtypes.bfloat16
except ImportError:
    _BF16 = None


def _to_bf16(a):
    """f32 -> bf16 (round-to-nearest-even); uint16 view fallback."""
    if _BF16 is not None:
        return a.astype(_BF16)
    x = np.ascontiguousarray(a, np.float32).view(np.uint32)
    r = ((x >> 16) & 1) + 0x7FFF
    return ((x + r) >> 16).astype(np.uint16)


N_CORES = 8
B, S, K = 64, 200, 50
V, E = 100000, 128
KE = K + 1            # items + bias slot
BIAS_V = V            # logical row V   = b
ZERO_V = V + 1        # logical row V+1 = 0
NV = V + 2            # logical vocab incl. bias+zero rows
P = 128
B_PER_CORE = B // N_CORES
BAGS = B_PER_CORE * S               # 1600 bags per core
N_TILES = -(-BAGS // P)             # 13
BAGS_PAD = N_TILES * P              # 1664

CHUNK_CAP = 32767                   # real rows per chunk (idx 1..32767)
N_CHUNKS = -(-NV // CHUNK_CAP)      # 4
# chunk 3 is mostly empty (1701 natural rows); fill its spare idx space with
# DUPLICATES of chunk-0..2 rows (chosen per call, by usefulness for shedding
# over-full bags) so bags can rebalance slots across chunks, smoothing the
# per-chunk counts that drive padding
N_NAT3 = NV - 3 * CHUNK_CAP         # 1701 natural chunk-3 rows
DUP_BASE = N_NAT3 + 1               # chunk-3 idx of first duplicate
N_DUP = CHUNK_CAP - DUP_BASE + 1    # 31066 duplicate slots
CHUNK_ROWS = [CHUNK_CAP + 1] * 3 + [DUP_BASE + N_DUP]


def _pick_dup_ids(all_bags):
    """Choose the N_DUP most useful rows to duplicate into chunk 3: score each
    id by its occurrences inside (bag, chunk) pairs that are over-full."""
    C = all_bags // CHUNK_CAP
    cnt = np.stack([(C == c).sum(axis=1) for c in range(N_CHUNKS)], axis=1)
    level = 13  # bags want per-chunk counts near ~12.75
    need = cnt > level                      # [bags, NC]
    useful = need[np.arange(len(all_bags))[:, None], C] & (all_bags < 3 * CHUNK_CAP)
    score = np.bincount(
        all_bags[useful].astype(np.int64), minlength=3 * CHUNK_CAP
    )
    dup_ids = np.sort(np.argsort(-score, kind="stable")[:N_DUP])
    dup_rank = np.full(3 * CHUNK_CAP, -1, np.int64)
    dup_rank[dup_ids] = np.arange(N_DUP)
    return dup_ids, dup_rank

TRACE = False       # test.py flips this to profile
LAST_RESULTS = {}   # test.py reads exec_time_ns etc. from here


def _build_tables(W, b, dup_ids):
    """bf16 chunk tables, each [zero row; <=CHUNK_CAP vocab rows].

    Separate tensors (not slices of one): the gather ucode's row addressing
    breaks when AP-base-offset + idx exceeds 32767 rows."""
    wt = np.zeros((NV, E), np.float32)
    wt[:V] = W.T
    wt[BIAS_V] = b
    tabs = []
    for c in range(3):
        t = np.zeros((CHUNK_ROWS[c], E), np.float32)
        t[1:] = wt[CHUNK_CAP * c : CHUNK_CAP * (c + 1)]
        tabs.append(_to_bf16(t))
    t3 = np.zeros((CHUNK_ROWS[3], E), np.float32)
    t3[1 : 1 + N_NAT3] = wt[3 * CHUNK_CAP :]
    t3[DUP_BASE:] = wt[dup_ids]
    tabs.append(_to_bf16(t3))
    return tabs


def _plan_core(v_bags, dup_rank):
    """v_bags: [BAGS, K] logical rows. Returns per-bag chunk-sorted idx lists
    and per-chunk counts, after rebalancing duplicate-eligible slots from
    over-full chunks 0..2 into chunk 3 to minimize each bag's max count.

    sorted_idx[bag, j] = local int16 idx of the bag's j-th slot when slots are
    ordered by (rebalanced) chunk; cnt[bag, c] = slots in chunk c."""
    C = v_bags // CHUNK_CAP                      # [BAGS, K] natural chunk
    elig = (v_bags < 3 * CHUNK_CAP) & (
        dup_rank[np.minimum(v_bags, 3 * CHUNK_CAP - 1)] >= 0
    )
    C2 = C.copy()
    for b in range(v_bags.shape[0]):
        row = C2[b]
        c = [(row == x).sum() for x in range(N_CHUNKS)]
        movable = [list(np.where((row == x) & elig[b])[0]) for x in range(3)]
        while True:
            moved = False
            for mx in sorted(range(3), key=lambda x: -c[x]):
                if c[mx] <= c[3] + 1:
                    break
                if movable[mx]:
                    row[movable[mx].pop()] = 3
                    c[mx] -= 1
                    c[3] += 1
                    moved = True
                    break
            if not moved:
                break
    IDX = np.where(
        C2 == C,
        v_bags - C * CHUNK_CAP + 1,
        DUP_BASE + dup_rank[np.minimum(v_bags, 3 * CHUNK_CAP - 1)],
    ).astype(np.int16)
    order = np.argsort(C2, axis=1, kind="stable")  # chunk-major slot order
    IDX_sorted = np.take_along_axis(IDX, order, axis=1)
    cnt = np.stack([(C2 == c).sum(axis=1) for c in range(N_CHUNKS)], axis=1)
    return IDX_sorted, cnt


# Per-tile pipeline cost (width units): desc-gen waits on the widest chunk's
# queue (128*7.57ns per width unit => 3.73x), the DVE reduce on the chunk-width
# sum (128*2.03ns per unit => 1x); the tile costs whichever engine is slower.
_COST_MAXC = 3.73


def _tile_cost(m):
    """m: [..., NC] per-chunk widths -> per-tile pipeline cost."""
    return np.maximum(_COST_MAXC * m.max(axis=-1), m.sum(axis=-1))


def _cluster_once(cnt, order):
    m = np.zeros((N_TILES, N_CHUNKS), np.int64)
    fill = np.zeros(N_TILES, np.int64)
    tiles = np.full((N_TILES, P), -1, np.int64)
    for b in order:
        best_key, best_t = None, None
        for t in range(N_TILES):
            if fill[t] >= P:
                continue
            nm = np.maximum(m[t], cnt[b])
            inc = float(_tile_cost(nm) - _tile_cost(m[t]))
            key = (inc, -int(fill[t]))
            if best_key is None or key < best_key:
                best_key, best_t = key, t
        tiles[best_t, fill[best_t]] = b
        m[best_t] = np.maximum(m[best_t], cnt[b])
        fill[best_t] += 1
    return tiles, m


def _refine(tiles, cnt_ext, iters=120):
    """Swap-based local search: repeatedly swap a bag out of the widest tile
    when it lowers the summed per-tile per-chunk maxima."""

    def tile_m(t):
        return cnt_ext[tiles[t]].max(axis=0)

    def max_without(members):
        """[P, NC] per-chunk max over members excluding each member."""
        ct = cnt_ext[members]
        srt = np.sort(ct, axis=0)
        top1, top2 = srt[-1], srt[-2]
        is_top = ct == top1[None, :]
        uniq = is_top.sum(axis=0) == 1
        return ct, np.where(is_top & uniq[None, :], top2[None, :], top1[None, :])

    m = np.stack([tile_m(t) for t in range(N_TILES)])
    for _ in range(iters):
        t = int(_tile_cost(m).argmax())
        ct, m_wo_t = max_without(tiles[t])
        others = [u for u in range(N_TILES) if u != t]
        cb_list, m_wo_list = zip(*(max_without(tiles[u]) for u in others))
        cb = np.concatenate(cb_list)           # [M, NC] candidate counts
        m_wo_u = np.concatenate(m_wo_list)     # [M, NC] u's width w/o candidate
        # widths of t after swapping member i with candidate j
        new_t = np.maximum(m_wo_t[:, None, :], cb[None, :, :])  # [P, M, NC]
        d_t = _tile_cost(new_t) - _tile_cost(m[t])
        # exact widths of u after losing candidate j and receiving member i
        new_u = np.maximum(m_wo_u[None, :, :], ct[:, None, :])  # [P, M, NC]
        u_costs = np.repeat(_tile_cost(m[others]), P)
        d_u = _tile_cost(new_u) - u_costs[None, :]
        delta = d_t + d_u
        i, j = np.unravel_index(int(delta.argmin()), delta.shape)
        if delta[i, j] >= -1e-9:
            break
        u_idx = others[j // P]  # j indexes (tile-in-others, slot)
        slot = j % P
        tiles[t][i], tiles[u_idx][slot] = tiles[u_idx][slot], tiles[t][i]
        m[t] = tile_m(t)
        m[u_idx] = tile_m(u_idx)
    return tiles, m


def _refine_balance(tiles, cnt_ext, iters=400):
    """Second local-search phase: minimize (max per-tile total width, sum),
    so the 13 tiles pipeline evenly (no heavy head-of-line tile)."""

    def tile_m(t):
        return cnt_ext[tiles[t]].max(axis=0)

    def max_without(members):
        ct = cnt_ext[members]
        srt = np.sort(ct, axis=0)
        top1, top2 = srt[-1], srt[-2]
        is_top = ct == top1[None, :]
        uniq = is_top.sum(axis=0) == 1
        return ct, np.where(is_top & uniq[None, :], top2[None, :], top1[None, :])

    m = np.stack([tile_m(t) for t in range(N_TILES)])
    for _ in range(iters):
        sw = _tile_cost(m)
        t = int(sw.argmax())
        ct, m_wo_t = max_without(tiles[t])
        others = [u for u in range(N_TILES) if u != t]
        cb_list, m_wo_list = zip(*(max_without(tiles[u]) for u in others))
        cb = np.concatenate(cb_list)
        m_wo_u = np.concatenate(m_wo_list)
        new_t_sw = _tile_cost(np.maximum(m_wo_t[:, None, :], cb[None, :, :]))
        new_u_sw = _tile_cost(np.maximum(m_wo_u[None, :, :], ct[:, None, :]))
        u_sw = np.repeat(sw[others], P)
        # sum-of-squares on cost: equalizes tiles without inflating the total
        score = (new_t_sw**2 + new_u_sw**2) - (sw[t] ** 2 + u_sw[None, :] ** 2)
        i, j = np.unravel_index(int(score.argmin()), score.shape)
        if score[i, j] >= -1e-9:
            break
        u_idx = others[j // P]
        slot = j % P
        tiles[t][i], tiles[u_idx][slot] = tiles[u_idx][slot], tiles[t][i]
        m[t] = tile_m(t)
        m[u_idx] = tile_m(u_idx)
    return tiles, m


def _cluster(cnt):
    """Greedy-pack 1600 bags into 13 tiles of 128 minimizing sum of per-tile
    per-chunk maxima; best of a few orderings. Returns tiles [N_TILES, P] of
    bag ids (-1 = dummy)."""
    orders = [
        np.argsort(-cnt.max(axis=1), kind="stable"),
        np.argsort(-cnt[:, :3].max(axis=1), kind="stable"),
        np.lexsort((cnt[:, 2], cnt[:, 1], cnt[:, 0]))[::-1],
    ]
    rng = np.random.default_rng(0)
    base = np.argsort(-cnt.max(axis=1), kind="stable")
    for _ in range(12):
        # perturbed difficulty order: keeps hard bags early but varies packing
        noise = rng.normal(0, 1.5, size=len(cnt))
        orders.append(np.argsort(-(cnt.max(axis=1) + noise), kind="stable"))
    for _ in range(3):
        orders.append(rng.permutation(len(cnt)))
    best = None
    for order in orders:
        tiles, m = _cluster_once(cnt, order)
        tot = float(_tile_cost(m).sum())
        if best is None or tot < best[0]:
            best = (tot, tiles, m)
    _, tiles, m = best
    # -1 dummies index the appended all-zeros row of cnt_ext
    cnt_ext = np.vstack([cnt, np.zeros((1, N_CHUNKS), cnt.dtype)])
    tiles, m = _refine(tiles, cnt_ext)
    tiles, m = _refine_balance(tiles, cnt_ext)
    # lightest tile first: fills the gather/reduce pipeline fastest
    order_t = np.argsort(_tile_cost(m), kind="stable")
    return tiles[order_t], m[order_t]


def _wrap_idxs(arr, w):
    """arr [P, w] int16 (partition-major slot grid) -> [128, w*8] wrapped+replicated."""
    L = P * w
    i = np.arange(L)
    lin = arr[i % P, i // P]                     # list position i = col*128 + p
    wrapped = lin.reshape(w * 8, 16).T           # [16, w*8]
    return np.tile(wrapped, (8, 1)).astype(np.int16)


def _prep_inputs(content_input, W, b):
    """Returns (in_maps, widths) where widths[t][c] is shared across cores."""
    ids = np.asarray(content_input).astype(np.int64).reshape(B, S, K)
    Wf = np.asarray(W, dtype=np.float32)
    bf = np.asarray(b, dtype=np.float32)

    ids = np.where(ids == 0, ZERO_V, ids)
    dup_ids, dup_rank = _pick_dup_ids(ids.reshape(B * S, K))
    tabs = _build_tables(Wf, bf, dup_ids)
    per_core = []
    tiles_per_core = []
    widths = np.zeros((N_TILES, N_CHUNKS), np.int64)
    for c in range(N_CORES):
        bag_ids = ids[c * B_PER_CORE : (c + 1) * B_PER_CORE].reshape(BAGS, K)
        plan = _plan_core(bag_ids, dup_rank)  # bias added on-device
        per_core.append(plan)
        tiles, m = _cluster(plan[1])
        tiles_per_core.append(tiles)
        widths = np.maximum(widths, m)

    bias_rep = np.ascontiguousarray(np.tile(bf[None, :], (P, 1)).astype(np.float32))
    in_maps = []
    for core in range(N_CORES):
        IDX_sorted, cnt = per_core[core]
        tiles = tiles_per_core[core]
        planes = []
        for t in range(N_TILES):
            bags = tiles[t]  # [P] bag ids, -1 = dummy
            real = bags >= 0
            bsafe = np.where(real, bags, 0)
            csum = np.zeros((P,), np.int64)
            for ch in range(N_CHUNKS):
                w = int(widths[t][ch])
                if w == 0:
                    continue
                cn = np.where(real, cnt[bsafe, ch], 0)
                j = np.arange(w)[None, :]
                src = np.take_along_axis(
                    IDX_sorted[bsafe], np.minimum(csum[:, None] + j, K - 1), axis=1
                )
                arr = np.where((j < cn[:, None]) & real[:, None], src, 0).astype(
                    np.int16
                )
                planes.append(_wrap_idxs(arr, w))
                csum += cn
        ids16 = np.concatenate(planes, axis=1)
        m = {"ids16": np.ascontiguousarray(ids16), "bias": bias_rep}
        for c in range(N_CHUNKS):
            m[f"wt{c}"] = tabs[c]
        in_maps.append(m)
    return in_maps, widths, tiles_per_core


def _build_program(widths, ids_cols):
    import concourse.bass as bass
    import concourse.mybir as mybir
    from concourse import bacc
    from concourse.tile import TileContext

    sw = [int(widths[t].sum()) for t in range(N_TILES)]
    sw_max = max(sw)
    # g tile is sw_max*256B per partition; keep total pool under ~160KB/partition
    g_bufs = max(1, min(6, (160 * 1024) // (sw_max * E * 2)))

    nc = bacc.Bacc("TRN2", target_bir_lowering=False, debug=False,
                   num_devices=N_CORES, num_swdge_queues=4)
    ids_d = nc.declare_dram_parameter("ids16", [P, ids_cols], mybir.dt.int16, isOutput=False)
    wt_ds = [
        nc.declare_dram_parameter(
            f"wt{c}", [CHUNK_ROWS[c], E], mybir.dt.bfloat16, isOutput=False
        )
        for c in range(N_CHUNKS)
    ]
    bias_d = nc.declare_dram_parameter("bias", [P, E], mybir.dt.float32, isOutput=False)
    out_d = nc.declare_dram_parameter("out", [BAGS_PAD, E], mybir.dt.float32, isOutput=True)

    with TileContext(nc) as tc:
        with (
            tc.tile_pool(name="ids", bufs=1) as ids_pool,
            tc.tile_pool(name="bias", bufs=1) as bias_pool,
            tc.tile_pool(name="g", bufs=g_bufs) as g_pool,
            tc.tile_pool(name="o", bufs=3) as o_pool,
        ):
            bias_sb = bias_pool.tile([P, E], mybir.dt.float32)
            nc.sync.dma_start(out=bias_sb[:], in_=bias_d[:])
            ids_sb = ids_pool.tile([P, ids_cols], mybir.dt.int16)
            nc.sync.dma_start(out=ids_sb[:], in_=ids_d[:])
            off = 0  # free-dim offset into ids16, in idx elements
            qi = 0   # emitted-gather counter; queue = qi % 4 keeps each of
            # Tile's 8 round-robin DMASW lanes locked to a single SWDGE queue
            for t in range(N_TILES):
                g = g_pool.tile([P, sw_max * E], mybir.dt.bfloat16)
                col = 0
                for ch in range(N_CHUNKS):
                    w = int(widths[t][ch])
                    if w == 0:
                        continue
                    n = w * P
                    dst = g[:, col * E : (col + w) * E].rearrange(
                        "p (j e) -> p j e", j=w, e=E
                    )
                    nc.gpsimd.dma_gather(
                        dst,
                        wt_ds[ch][:],
                        ids_sb[:, off : off + w * 8],
                        n,
                        n,
                        E,
                        single_packet=False,
                        queue_num=qi % 4,
                    )
                    qi += 1
                    col += w
                    off += w * 8
                o = o_pool.tile([P, E], mybir.dt.float32)
                nc.vector.tensor_reduce(
                    out=o[:],
                    in_=g[:, : sw[t] * E].rearrange("p (j e) -> p e j", j=sw[t], e=E),
                    axis=mybir.AxisListType.X,
                    op=mybir.AluOpType.add,
                )
                nc.vector.tensor_add(out=o[:], in0=o[:], in1=bias_sb[:])
                nc.sync.dma_start(out=out_d[t * P : (t + 1) * P, :], in_=o[:])
    nc.compile()
    return nc


def kernel(content_input, W, b):
    global LAST_RESULTS
    in_maps, widths, tiles_per_core = _prep_inputs(content_input, W, b)
    ids_cols = in_maps[0]["ids16"].shape[1]
    nc = _build_program(widths, ids_cols)

    from concourse.bass_utils import run_bass_kernel_spmd

    res = run_bass_kernel_spmd(nc, in_maps, list(range(N_CORES)), trace=TRACE)
    LAST_RESULTS = {
        "exec_time_ns": res.exec_time_ns,
        "mean_exec_time_ns": res.mean_exec_time_ns,
        "instructions_and_trace": res.instructions_and_trace,
        "profile_json": res.profile_json,
        "widths": widths,
    }

    out = np.empty((B, S, E), np.float32)
    for c in range(N_CORES):
        rows = res.results[c]["out"]  # [BAGS_PAD, E], permuted bag order
        flat = np.empty((BAGS, E), np.float32)
        tiles = tiles_per_core[c].reshape(BAGS_PAD)
        real = tiles >= 0
        flat[tiles[real]] = rows[real]
        out[c * B_PER_CORE : (c + 1) * B_PER_CORE] = flat.reshape(B_PER_CORE, S, E)
    return out



# revision 22
# speedup vs baseline: 2.7145x; 1.0124x over previous
"""Trainium2 Bass kernel for nn_LinearUpscaler (masked embedding-bag sum + bias).

reference:  g = W.T[ids]; g[ids == 0] = 0; out = g.sum(axis=2) + b

Design: data-parallel over batch across 8 cores (8 batch rows -> 1600 bags of
51 slots each: 50 items + 1 bias-row slot; ids==0 remapped to a zero row).

The gather engine is the GPSIMD dma_gather custom op (int16 indices, one
512B/256B row per index, written to partition i%128, column i//128).  Since
indices are signed int16 (max 32767) and V=100002, the fp16 table is split
into 4 vocab chunks; row 0 of each chunk is a zero row so padding slots can
gather harmlessly.  For each tile of 128 bags, each bag's slots are bucketed
by chunk; per (tile, chunk) all bags are padded to a common width W so the
dma_gather index list is fully valid (no negative indices) with a
compile-time num_idxs.  The list order is chosen so bag b's rows land in
partition b at consecutive columns; one strided vector-engine reduce per tile
sums items+chunks+padding (pads contribute zeros) in a single pass, and the
result is DMA'd out.  The program is specialized per call (widths depend on
the actual ids); no collectives are needed.
"""

import importlib.util
import os
import sys

if importlib.util.find_spec("concourse") is None:
    for _p in ("/opt/trn_rl_repo", "/root/.axon_site/_ro/trn_rl_repo"):
        if os.path.isdir(_p) and _p not in sys.path:
            sys.path.insert(0, _p)
            break

import numpy as np

try:
    import ml_dtypes
    _BF16 = ml_dtypes.bfloat16
except ImportError:
    _BF16 = None


def _to_bf16(a):
    """f32 -> bf16 (round-to-nearest-even); uint16 view fallback."""
    if _BF16 is not None:
        return a.astype(_BF16)
    x = np.ascontiguousarray(a, np.float32).view(np.uint32)
    r = ((x >> 16) & 1) + 0x7FFF
    return ((x + r) >> 16).astype(np.uint16)


N_CORES = 8
B, S, K = 64, 200, 50
V, E = 100000, 128
KE = K + 1            # items + bias slot
BIAS_V = V            # logical row V   = b
ZERO_V = V + 1        # logical row V+1 = 0
NV = V + 2            # logical vocab incl. bias+zero rows
P = 128
B_PER_CORE = B // N_CORES
BAGS = B_PER_CORE * S               # 1600 bags per core
N_TILES = -(-BAGS // P)             # 13
BAGS_PAD = N_TILES * P              # 1664

CHUNK_CAP = 32767                   # real rows per chunk (idx 1..32767)
N_CHUNKS = -(-NV // CHUNK_CAP)      # 4
# chunk 3 is mostly empty (1701 natural rows); fill its spare idx space with
# DUPLICATES of chunk-0..2 rows (chosen per call, by usefulness for shedding
# over-full bags) so bags can rebalance slots across chunks, smoothing the
# per-chunk counts that drive padding
N_NAT3 = NV - 3 * CHUNK_CAP         # 1701 natural chunk-3 rows
DUP_BASE = N_NAT3 + 1               # chunk-3 idx of first duplicate
N_DUP = CHUNK_CAP - DUP_BASE + 1    # 31066 duplicate slots
CHUNK_ROWS = [CHUNK_CAP + 1] * 3 + [DUP_BASE + N_DUP]


def _pick_dup_ids(all_bags, level=13, extra_score=None):
    """Choose the N_DUP most useful rows to duplicate into chunk 3: score each
    id by its occurrences inside (bag, chunk) pairs that are over-full."""
    C = all_bags // CHUNK_CAP
    cnt = np.stack([(C == c).sum(axis=1) for c in range(N_CHUNKS)], axis=1)
    need = cnt > level                      # [bags, NC]
    useful = need[np.arange(len(all_bags))[:, None], C] & (all_bags < 3 * CHUNK_CAP)
    score = np.bincount(
        all_bags[useful].astype(np.int64), minlength=3 * CHUNK_CAP
    ).astype(np.float64)
    if extra_score is not None:
        score += extra_score
    dup_ids = np.sort(np.argsort(-score, kind="stable")[:N_DUP])
    dup_rank = np.full(3 * CHUNK_CAP, -1, np.int64)
    dup_rank[dup_ids] = np.arange(N_DUP)
    return dup_ids, dup_rank

TRACE = False       # test.py flips this to profile
LAST_RESULTS = {}   # test.py reads exec_time_ns etc. from here


def _build_tables(W, b, dup_ids):
    """bf16 chunk tables, each [zero row; <=CHUNK_CAP vocab rows].

    Separate tensors (not slices of one): the gather ucode's row addressing
    breaks when AP-base-offset + idx exceeds 32767 rows."""
    wt = np.zeros((NV, E), np.float32)
    wt[:V] = W.T
    wt[BIAS_V] = b
    tabs = []
    for c in range(3):
        t = np.zeros((CHUNK_ROWS[c], E), np.float32)
        t[1:] = wt[CHUNK_CAP * c : CHUNK_CAP * (c + 1)]
        tabs.append(_to_bf16(t))
    t3 = np.zeros((CHUNK_ROWS[3], E), np.float32)
    t3[1 : 1 + N_NAT3] = wt[3 * CHUNK_CAP :]
    t3[DUP_BASE:] = wt[dup_ids]
    tabs.append(_to_bf16(t3))
    return tabs


def _plan_core(v_bags, dup_rank):
    """v_bags: [BAGS, K] logical rows. Returns per-bag chunk-sorted idx lists
    and per-chunk counts, after rebalancing duplicate-eligible slots from
    over-full chunks 0..2 into chunk 3 to minimize each bag's max count.

    sorted_idx[bag, j] = local int16 idx of the bag's j-th slot when slots are
    ordered by (rebalanced) chunk; cnt[bag, c] = slots in chunk c."""
    C = v_bags // CHUNK_CAP                      # [BAGS, K] natural chunk
    elig = (v_bags < 3 * CHUNK_CAP) & (
        dup_rank[np.minimum(v_bags, 3 * CHUNK_CAP - 1)] >= 0
    )
    C2 = C.copy()
    for b in range(v_bags.shape[0]):
        row = C2[b]
        c = [(row == x).sum() for x in range(N_CHUNKS)]
        movable = [list(np.where((row == x) & elig[b])[0]) for x in range(3)]
        while True:
            moved = False
            for mx in sorted(range(3), key=lambda x: -c[x]):
                if c[mx] <= c[3] + 1:
                    break
                if movable[mx]:
                    row[movable[mx].pop()] = 3
                    c[mx] -= 1
                    c[3] += 1
                    moved = True
                    break
            if not moved:
                break
    IDX = np.where(
        C2 == C,
        v_bags - C * CHUNK_CAP + 1,
        DUP_BASE + dup_rank[np.minimum(v_bags, 3 * CHUNK_CAP - 1)],
    ).astype(np.int16)
    order = np.argsort(C2, axis=1, kind="stable")  # chunk-major slot order
    IDX_sorted = np.take_along_axis(IDX, order, axis=1)
    cnt = np.stack([(C2 == c).sum(axis=1) for c in range(N_CHUNKS)], axis=1)
    return IDX_sorted, cnt


# Per-tile pipeline cost (width units): desc-gen waits on the widest chunk's
# queue (128*7.57ns per width unit => 3.73x), the DVE reduce on the chunk-width
# sum (128*2.03ns per unit => 1x); the tile costs whichever engine is slower.
_COST_MAXC = 3.73


def _tile_cost(m):
    """m: [..., NC] per-chunk widths -> per-tile pipeline cost."""
    return np.maximum(_COST_MAXC * m.max(axis=-1), m.sum(axis=-1))


def _cluster_once(cnt, order):
    m = np.zeros((N_TILES, N_CHUNKS), np.int64)
    fill = np.zeros(N_TILES, np.int64)
    tiles = np.full((N_TILES, P), -1, np.int64)
    for b in order:
        best_key, best_t = None, None
        for t in range(N_TILES):
            if fill[t] >= P:
                continue
            nm = np.maximum(m[t], cnt[b])
            inc = float(_tile_cost(nm) - _tile_cost(m[t]))
            key = (inc, -int(fill[t]))
            if best_key is None or key < best_key:
                best_key, best_t = key, t
        tiles[best_t, fill[best_t]] = b
        m[best_t] = np.maximum(m[best_t], cnt[b])
        fill[best_t] += 1
    return tiles, m


def _refine(tiles, cnt_ext, iters=120):
    """Swap-based local search: repeatedly swap a bag out of the widest tile
    when it lowers the summed per-tile per-chunk maxima."""

    def tile_m(t):
        return cnt_ext[tiles[t]].max(axis=0)

    def max_without(members):
        """[P, NC] per-chunk max over members excluding each member."""
        ct = cnt_ext[members]
        srt = np.sort(ct, axis=0)
        top1, top2 = srt[-1], srt[-2]
        is_top = ct == top1[None, :]
        uniq = is_top.sum(axis=0) == 1
        return ct, np.where(is_top & uniq[None, :], top2[None, :], top1[None, :])

    m = np.stack([tile_m(t) for t in range(N_TILES)])
    for _ in range(iters):
        t = int(_tile_cost(m).argmax())
        ct, m_wo_t = max_without(tiles[t])
        others = [u for u in range(N_TILES) if u != t]
        cb_list, m_wo_list = zip(*(max_without(tiles[u]) for u in others))
        cb = np.concatenate(cb_list)           # [M, NC] candidate counts
        m_wo_u = np.concatenate(m_wo_list)     # [M, NC] u's width w/o candidate
        # widths of t after swapping member i with candidate j
        new_t = np.maximum(m_wo_t[:, None, :], cb[None, :, :])  # [P, M, NC]
        d_t = _tile_cost(new_t) - _tile_cost(m[t])
        # exact widths of u after losing candidate j and receiving member i
        new_u = np.maximum(m_wo_u[None, :, :], ct[:, None, :])  # [P, M, NC]
        u_costs = np.repeat(_tile_cost(m[others]), P)
        d_u = _tile_cost(new_u) - u_costs[None, :]
        delta = d_t + d_u
        i, j = np.unravel_index(int(delta.argmin()), delta.shape)
        if delta[i, j] >= -1e-9:
            break
        u_idx = others[j // P]  # j indexes (tile-in-others, slot)
        slot = j % P
        tiles[t][i], tiles[u_idx][slot] = tiles[u_idx][slot], tiles[t][i]
        m[t] = tile_m(t)
        m[u_idx] = tile_m(u_idx)
    return tiles, m


def _refine_balance(tiles, cnt_ext, iters=400):
    """Second local-search phase: minimize (max per-tile total width, sum),
    so the 13 tiles pipeline evenly (no heavy head-of-line tile)."""

    def tile_m(t):
        return cnt_ext[tiles[t]].max(axis=0)

    def max_without(members):
        ct = cnt_ext[members]
        srt = np.sort(ct, axis=0)
        top1, top2 = srt[-1], srt[-2]
        is_top = ct == top1[None, :]
        uniq = is_top.sum(axis=0) == 1
        return ct, np.where(is_top & uniq[None, :], top2[None, :], top1[None, :])

    m = np.stack([tile_m(t) for t in range(N_TILES)])
    for _ in range(iters):
        sw = _tile_cost(m)
        t = int(sw.argmax())
        ct, m_wo_t = max_without(tiles[t])
        others = [u for u in range(N_TILES) if u != t]
        cb_list, m_wo_list = zip(*(max_without(tiles[u]) for u in others))
        cb = np.concatenate(cb_list)
        m_wo_u = np.concatenate(m_wo_list)
        new_t_sw = _tile_cost(np.maximum(m_wo_t[:, None, :], cb[None, :, :]))
        new_u_sw = _tile_cost(np.maximum(m_wo_u[None, :, :], ct[:, None, :]))
        u_sw = np.repeat(sw[others], P)
        # sum-of-squares on cost: equalizes tiles without inflating the total
        score = (new_t_sw**2 + new_u_sw**2) - (sw[t] ** 2 + u_sw[None, :] ** 2)
        i, j = np.unravel_index(int(score.argmin()), score.shape)
        if score[i, j] >= -1e-9:
            break
        u_idx = others[j // P]
        slot = j % P
        tiles[t][i], tiles[u_idx][slot] = tiles[u_idx][slot], tiles[t][i]
        m[t] = tile_m(t)
        m[u_idx] = tile_m(u_idx)
    return tiles, m


def _cluster(cnt):
    """Greedy-pack 1600 bags into 13 tiles of 128 minimizing sum of per-tile
    per-chunk maxima; best of a few orderings. Returns tiles [N_TILES, P] of
    bag ids (-1 = dummy)."""
    orders = [
        np.argsort(-cnt.max(axis=1), kind="stable"),
        np.argsort(-cnt[:, :3].max(axis=1), kind="stable"),
        np.lexsort((cnt[:, 2], cnt[:, 1], cnt[:, 0]))[::-1],
    ]
    rng = np.random.default_rng(0)
    base = np.argsort(-cnt.max(axis=1), kind="stable")
    for _ in range(12):
        # perturbed difficulty order: keeps hard bags early but varies packing
        noise = rng.normal(0, 1.5, size=len(cnt))
        orders.append(np.argsort(-(cnt.max(axis=1) + noise), kind="stable"))
    for _ in range(3):
        orders.append(rng.permutation(len(cnt)))
    best = None
    for order in orders:
        tiles, m = _cluster_once(cnt, order)
        tot = float(_tile_cost(m).sum())
        if best is None or tot < best[0]:
            best = (tot, tiles, m)
    _, tiles, m = best
    # -1 dummies index the appended all-zeros row of cnt_ext
    cnt_ext = np.vstack([cnt, np.zeros((1, N_CHUNKS), cnt.dtype)])
    tiles, m = _refine(tiles, cnt_ext)
    tiles, m = _refine_balance(tiles, cnt_ext)
    # pyramid order (light tiles at both ends): a light head fills the
    # gather/reduce pipeline fast, a light tail drains the DMA backlog fast
    asc = np.argsort(_tile_cost(m), kind="stable")
    order_t = np.concatenate([asc[0::2], asc[1::2][::-1]])
    return tiles[order_t], m[order_t]


def _wrap_idxs(arr, w):
    """arr [P, w] int16 (partition-major slot grid) -> [128, w*8] wrapped+replicated."""
    L = P * w
    i = np.arange(L)
    lin = arr[i % P, i // P]                     # list position i = col*128 + p
    wrapped = lin.reshape(w * 8, 16).T           # [16, w*8]
    return np.tile(wrapped, (8, 1)).astype(np.int16)


def _prep_inputs(content_input, W, b):
    """Returns (in_maps, widths) where widths[t][c] is shared across cores."""
    ids = np.asarray(content_input).astype(np.int64).reshape(B, S, K)
    Wf = np.asarray(W, dtype=np.float32)
    bf = np.asarray(b, dtype=np.float32)

    ids = np.where(ids == 0, ZERO_V, ids)
    all_bags = ids.reshape(B * S, K)
    dup_ids, dup_rank = _pick_dup_ids(all_bags)
    tabs = _build_tables(Wf, bf, dup_ids)
    per_core = []
    tiles_per_core = []
    widths = np.zeros((N_TILES, N_CHUNKS), np.int64)
    for c in range(N_CORES):
        bag_ids = ids[c * B_PER_CORE : (c + 1) * B_PER_CORE].reshape(BAGS, K)
        plan = _plan_core(bag_ids, dup_rank)  # bias added on-device
        per_core.append(plan)
        tiles, m = _cluster(plan[1])
        tiles_per_core.append(tiles)
        widths = np.maximum(widths, m)

    bias_rep = np.ascontiguousarray(np.tile(bf[None, :], (P, 1)).astype(np.float32))
    in_maps = []
    for core in range(N_CORES):
        IDX_sorted, cnt = per_core[core]
        tiles = tiles_per_core[core]
        planes = []
        for t in range(N_TILES):
            bags = tiles[t]  # [P] bag ids, -1 = dummy
            real = bags >= 0
            bsafe = np.where(real, bags, 0)
            csum = np.zeros((P,), np.int64)
            for ch in range(N_CHUNKS):
                w = int(widths[t][ch])
                if w == 0:
                    continue
                cn = np.where(real, cnt[bsafe, ch], 0)
                j = np.arange(w)[None, :]
                src = np.take_along_axis(
                    IDX_sorted[bsafe], np.minimum(csum[:, None] + j, K - 1), axis=1
                )
                arr = np.where((j < cn[:, None]) & real[:, None], src, 0).astype(
                    np.int16
                )
                planes.append(_wrap_idxs(arr, w))
                csum += cn
        ids16 = np.concatenate(planes, axis=1)
        m = {"ids16": np.ascontiguousarray(ids16), "bias": bias_rep}
        for c in range(N_CHUNKS):
            m[f"wt{c}"] = tabs[c]
        in_maps.append(m)
    return in_maps, widths, tiles_per_core


def _build_program(widths, ids_cols):
    import concourse.bass as bass
    import concourse.mybir as mybir
    from concourse import bacc
    from concourse.tile import TileContext

    sw = [int(widths[t].sum()) for t in range(N_TILES)]
    sw_max = max(sw)
    # g tile is sw_max*256B per partition; keep total pool under ~160KB/partition
    g_bufs = max(1, min(6, (160 * 1024) // (sw_max * E * 2)))

    nc = bacc.Bacc("TRN2", target_bir_lowering=False, debug=False,
                   num_devices=N_CORES, num_swdge_queues=4)
    ids_d = nc.declare_dram_parameter("ids16", [P, ids_cols], mybir.dt.int16, isOutput=False)
    wt_ds = [
        nc.declare_dram_parameter(
            f"wt{c}", [CHUNK_ROWS[c], E], mybir.dt.bfloat16, isOutput=False
        )
        for c in range(N_CHUNKS)
    ]
    bias_d = nc.declare_dram_parameter("bias", [P, E], mybir.dt.float32, isOutput=False)
    out_d = nc.declare_dram_parameter("out", [BAGS_PAD, E], mybir.dt.float32, isOutput=True)

    with TileContext(nc) as tc:
        with (
            tc.tile_pool(name="ids", bufs=1) as ids_pool,
            tc.tile_pool(name="bias", bufs=1) as bias_pool,
            tc.tile_pool(name="g", bufs=g_bufs) as g_pool,
            tc.tile_pool(name="o", bufs=3) as o_pool,
        ):
            bias_sb = bias_pool.tile([P, E], mybir.dt.float32)
            nc.sync.dma_start(out=bias_sb[:], in_=bias_d[:])
            # split the ids load so tile 0's gathers only wait for its slice
            t0_cols = int(widths[0].sum()) * 8
            ids_sb = ids_pool.tile([P, t0_cols], mybir.dt.int16)
            nc.sync.dma_start(out=ids_sb[:], in_=ids_d[:, :t0_cols])
            ids_sb2 = ids_pool.tile([P, ids_cols - t0_cols], mybir.dt.int16)
            nc.sync.dma_start(out=ids_sb2[:], in_=ids_d[:, t0_cols:])
            off = 0  # free-dim offset into ids16, in idx elements
            qi = 0   # emitted-gather counter; queue = qi % 4 keeps each of
            # Tile's 8 round-robin DMASW lanes locked to a single SWDGE queue
            for t in range(N_TILES):
                g = g_pool.tile([P, sw_max * E], mybir.dt.bfloat16)
                col = 0
                for ch in range(N_CHUNKS):
                    w = int(widths[t][ch])
                    if w == 0:
                        continue
                    n = w * P
                    dst = g[:, col * E : (col + w) * E].rearrange(
                        "p (j e) -> p j e", j=w, e=E
                    )
                    if off < t0_cols:
                        idx_ap = ids_sb[:, off : off + w * 8]
                    else:
                        idx_ap = ids_sb2[:, off - t0_cols : off - t0_cols + w * 8]
                    nc.gpsimd.dma_gather(
                        dst,
                        wt_ds[ch][:],
                        idx_ap,
                        n,
                        n,
                        E,
                        single_packet=False,
                        queue_num=qi % 4,
                    )
                    qi += 1
                    col += w
                    off += w * 8
                o = o_pool.tile([P, E], mybir.dt.float32)
                nc.vector.tensor_reduce(
                    out=o[:],
                    in_=g[:, : sw[t] * E].rearrange("p (j e) -> p e j", j=sw[t], e=E),
                    axis=mybir.AxisListType.X,
                    op=mybir.AluOpType.add,
                )
                nc.vector.tensor_add(out=o[:], in0=o[:], in1=bias_sb[:])
                nc.sync.dma_start(out=out_d[t * P : (t + 1) * P, :], in_=o[:])
    nc.compile()
    return nc


def kernel(content_input, W, b):
    global LAST_RESULTS
    in_maps, widths, tiles_per_core = _prep_inputs(content_input, W, b)
    ids_cols = in_maps[0]["ids16"].shape[1]
    nc = _build_program(widths, ids_cols)

    from concourse.bass_utils import run_bass_kernel_spmd

    res = run_bass_kernel_spmd(nc, in_maps, list(range(N_CORES)), trace=TRACE)
    LAST_RESULTS = {
        "exec_time_ns": res.exec_time_ns,
        "mean_exec_time_ns": res.mean_exec_time_ns,
        "instructions_and_trace": res.instructions_and_trace,
        "profile_json": res.profile_json,
        "widths": widths,
    }

    out = np.empty((B, S, E), np.float32)
    for c in range(N_CORES):
        rows = res.results[c]["out"]  # [BAGS_PAD, E], permuted bag order
        flat = np.empty((BAGS, E), np.float32)
        tiles = tiles_per_core[c].reshape(BAGS_PAD)
        real = tiles >= 0
        flat[tiles[real]] = rows[real]
        out[c * B_PER_CORE : (c + 1) * B_PER_CORE] = flat.reshape(B_PER_CORE, S, E)
    return out



# revision 25
# speedup vs baseline: 2.8497x; 1.0498x over previous
"""Trainium2 Bass kernel for nn_LinearUpscaler (masked embedding-bag sum + bias).

reference:  g = W.T[ids]; g[ids == 0] = 0; out = g.sum(axis=2) + b

Design: data-parallel over batch across 8 cores (8 batch rows -> 1600 bags of
51 slots each: 50 items + 1 bias-row slot; ids==0 remapped to a zero row).

The gather engine is the GPSIMD dma_gather custom op (int16 indices, one
512B/256B row per index, written to partition i%128, column i//128).  Since
indices are signed int16 (max 32767) and V=100002, the fp16 table is split
into 4 vocab chunks; row 0 of each chunk is a zero row so padding slots can
gather harmlessly.  For each tile of 128 bags, each bag's slots are bucketed
by chunk; per (tile, chunk) all bags are padded to a common width W so the
dma_gather index list is fully valid (no negative indices) with a
compile-time num_idxs.  The list order is chosen so bag b's rows land in
partition b at consecutive columns; one strided vector-engine reduce per tile
sums items+chunks+padding (pads contribute zeros) in a single pass, and the
result is DMA'd out.  The program is specialized per call (widths depend on
the actual ids); no collectives are needed.
"""

import importlib.util
import os
import sys

if importlib.util.find_spec("concourse") is None:
    for _p in ("/opt/trn_rl_repo", "/root/.axon_site/_ro/trn_rl_repo"):
        if os.path.isdir(_p) and _p not in sys.path:
            sys.path.insert(0, _p)
            break

import numpy as np

try:
    import ml_dtypes
    _BF16 = ml_dtypes.bfloat16
except ImportError:
    _BF16 = None


def _to_bf16(a):
    """f32 -> bf16 (round-to-nearest-even); uint16 view fallback."""
    if _BF16 is not None:
        return a.astype(_BF16)
    x = np.ascontiguousarray(a, np.float32).view(np.uint32)
    r = ((x >> 16) & 1) + 0x7FFF
    return ((x + r) >> 16).astype(np.uint16)


N_CORES = 8
B, S, K = 64, 200, 50
V, E = 100000, 128
KE = K + 1            # items + bias slot
BIAS_V = V            # logical row V   = b
ZERO_V = V + 1        # logical row V+1 = 0
NV = V + 2            # logical vocab incl. bias+zero rows
P = 128
B_PER_CORE = B // N_CORES
BAGS = B_PER_CORE * S               # 1600 bags per core
N_TILES = -(-BAGS // P)             # 13
BAGS_PAD = N_TILES * P              # 1664

CHUNK_CAP = 32767                   # real rows per chunk (idx 1..32767)
N_CHUNKS = -(-NV // CHUNK_CAP)      # 4
# chunk 3 is mostly empty (1701 natural rows); fill its spare idx space with
# DUPLICATES of chunk-0..2 rows (chosen per call, by usefulness for shedding
# over-full bags) so bags can rebalance slots across chunks, smoothing the
# per-chunk counts that drive padding
N_NAT3 = NV - 3 * CHUNK_CAP         # 1701 natural chunk-3 rows
DUP_BASE = N_NAT3 + 1               # chunk-3 idx of first duplicate
N_DUP = CHUNK_CAP - DUP_BASE + 1    # 31066 duplicate slots
CHUNK_ROWS = [CHUNK_CAP + 1] * 3 + [DUP_BASE + N_DUP]


def _pick_dup_ids(all_bags, level=13, extra_score=None):
    """Choose the N_DUP most useful rows to duplicate into chunk 3: score each
    id by its occurrences inside (bag, chunk) pairs that are over-full."""
    C = all_bags // CHUNK_CAP
    cnt = np.stack([(C == c).sum(axis=1) for c in range(N_CHUNKS)], axis=1)
    need = cnt > level                      # [bags, NC]
    useful = need[np.arange(len(all_bags))[:, None], C] & (all_bags < 3 * CHUNK_CAP)
    score = np.bincount(
        all_bags[useful].astype(np.int64), minlength=3 * CHUNK_CAP
    ).astype(np.float64)
    if extra_score is not None:
        score += extra_score
    dup_ids = np.sort(np.argsort(-score, kind="stable")[:N_DUP])
    dup_rank = np.full(3 * CHUNK_CAP, -1, np.int64)
    dup_rank[dup_ids] = np.arange(N_DUP)
    return dup_ids, dup_rank

TRACE = False       # test.py flips this to profile
LAST_RESULTS = {}   # test.py reads exec_time_ns etc. from here


def _build_tables(W, b, dup_ids):
    """bf16 chunk tables, each [zero row; <=CHUNK_CAP vocab rows].

    Separate tensors (not slices of one): the gather ucode's row addressing
    breaks when AP-base-offset + idx exceeds 32767 rows."""
    wt = np.zeros((NV, E), np.float32)
    wt[:V] = W.T
    wt[BIAS_V] = b
    tabs = []
    for c in range(3):
        t = np.zeros((CHUNK_ROWS[c], E), np.float32)
        t[1:] = wt[CHUNK_CAP * c : CHUNK_CAP * (c + 1)]
        tabs.append(_to_bf16(t))
    t3 = np.zeros((CHUNK_ROWS[3], E), np.float32)
    t3[1 : 1 + N_NAT3] = wt[3 * CHUNK_CAP :]
    t3[DUP_BASE:] = wt[dup_ids]
    tabs.append(_to_bf16(t3))
    return tabs


def _plan_core(v_bags, dup_rank):
    """v_bags: [BAGS, K] logical rows. Returns per-bag chunk-sorted idx lists
    and per-chunk counts, after rebalancing duplicate-eligible slots from
    over-full chunks 0..2 into chunk 3 to minimize each bag's max count.

    sorted_idx[bag, j] = local int16 idx of the bag's j-th slot when slots are
    ordered by (rebalanced) chunk; cnt[bag, c] = slots in chunk c."""
    C = v_bags // CHUNK_CAP                      # [BAGS, K] natural chunk
    elig = (v_bags < 3 * CHUNK_CAP) & (
        dup_rank[np.minimum(v_bags, 3 * CHUNK_CAP - 1)] >= 0
    )
    C2 = C.copy()
    for b in range(v_bags.shape[0]):
        row = C2[b]
        c = [(row == x).sum() for x in range(N_CHUNKS)]
        movable = [list(np.where((row == x) & elig[b])[0]) for x in range(3)]
        while True:
            moved = False
            for mx in sorted(range(3), key=lambda x: -c[x]):
                if c[mx] <= c[3] + 1:
                    break
                if movable[mx]:
                    row[movable[mx].pop()] = 3
                    c[mx] -= 1
                    c[3] += 1
                    moved = True
                    break
            if not moved:
                break
    IDX = np.where(
        C2 == C,
        v_bags - C * CHUNK_CAP + 1,
        DUP_BASE + dup_rank[np.minimum(v_bags, 3 * CHUNK_CAP - 1)],
    ).astype(np.int16)
    order = np.argsort(C2, axis=1, kind="stable")  # chunk-major slot order
    IDX_sorted = np.take_along_axis(IDX, order, axis=1)
    cnt = np.stack([(C2 == c).sum(axis=1) for c in range(N_CHUNKS)], axis=1)
    return IDX_sorted, cnt


# Per-tile pipeline cost (width units): desc-gen waits on the widest chunk's
# queue (128*7.57ns per width unit => 3.73x), the DVE reduce on the chunk-width
# sum (128*2.03ns per unit => 1x); the tile costs whichever engine is slower.
_COST_MAXC = 3.73


def _tile_cost(m):
    """m: [..., NC] per-chunk widths -> per-tile pipeline cost."""
    return np.maximum(_COST_MAXC * m.max(axis=-1), m.sum(axis=-1))


def _cluster_once(cnt, order):
    m = np.zeros((N_TILES, N_CHUNKS), np.int64)
    fill = np.zeros(N_TILES, np.int64)
    tiles = np.full((N_TILES, P), -1, np.int64)
    for b in order:
        best_key, best_t = None, None
        for t in range(N_TILES):
            if fill[t] >= P:
                continue
            nm = np.maximum(m[t], cnt[b])
            inc = float(_tile_cost(nm) - _tile_cost(m[t]))
            key = (inc, -int(fill[t]))
            if best_key is None or key < best_key:
                best_key, best_t = key, t
        tiles[best_t, fill[best_t]] = b
        m[best_t] = np.maximum(m[best_t], cnt[b])
        fill[best_t] += 1
    return tiles, m


def _refine(tiles, cnt_ext, iters=120):
    """Swap-based local search: repeatedly swap a bag out of the widest tile
    when it lowers the summed per-tile per-chunk maxima."""

    def tile_m(t):
        return cnt_ext[tiles[t]].max(axis=0)

    def max_without(members):
        """[P, NC] per-chunk max over members excluding each member."""
        ct = cnt_ext[members]
        srt = np.sort(ct, axis=0)
        top1, top2 = srt[-1], srt[-2]
        is_top = ct == top1[None, :]
        uniq = is_top.sum(axis=0) == 1
        return ct, np.where(is_top & uniq[None, :], top2[None, :], top1[None, :])

    m = np.stack([tile_m(t) for t in range(N_TILES)])
    for _ in range(iters):
        t = int(_tile_cost(m).argmax())
        ct, m_wo_t = max_without(tiles[t])
        others = [u for u in range(N_TILES) if u != t]
        cb_list, m_wo_list = zip(*(max_without(tiles[u]) for u in others))
        cb = np.concatenate(cb_list)           # [M, NC] candidate counts
        m_wo_u = np.concatenate(m_wo_list)     # [M, NC] u's width w/o candidate
        # widths of t after swapping member i with candidate j
        new_t = np.maximum(m_wo_t[:, None, :], cb[None, :, :])  # [P, M, NC]
        d_t = _tile_cost(new_t) - _tile_cost(m[t])
        # exact widths of u after losing candidate j and receiving member i
        new_u = np.maximum(m_wo_u[None, :, :], ct[:, None, :])  # [P, M, NC]
        u_costs = np.repeat(_tile_cost(m[others]), P)
        d_u = _tile_cost(new_u) - u_costs[None, :]
        delta = d_t + d_u
        i, j = np.unravel_index(int(delta.argmin()), delta.shape)
        if delta[i, j] >= -1e-9:
            break
        u_idx = others[j // P]  # j indexes (tile-in-others, slot)
        slot = j % P
        tiles[t][i], tiles[u_idx][slot] = tiles[u_idx][slot], tiles[t][i]
        m[t] = tile_m(t)
        m[u_idx] = tile_m(u_idx)
    return tiles, m


def _refine_balance(tiles, cnt_ext, iters=400):
    """Second local-search phase: minimize (max per-tile total width, sum),
    so the 13 tiles pipeline evenly (no heavy head-of-line tile)."""

    def tile_m(t):
        return cnt_ext[tiles[t]].max(axis=0)

    def max_without(members):
        ct = cnt_ext[members]
        srt = np.sort(ct, axis=0)
        top1, top2 = srt[-1], srt[-2]
        is_top = ct == top1[None, :]
        uniq = is_top.sum(axis=0) == 1
        return ct, np.where(is_top & uniq[None, :], top2[None, :], top1[None, :])

    m = np.stack([tile_m(t) for t in range(N_TILES)])
    for _ in range(iters):
        sw = _tile_cost(m)
        t = int(sw.argmax())
        ct, m_wo_t = max_without(tiles[t])
        others = [u for u in range(N_TILES) if u != t]
        cb_list, m_wo_list = zip(*(max_without(tiles[u]) for u in others))
        cb = np.concatenate(cb_list)
        m_wo_u = np.concatenate(m_wo_list)
        new_t_sw = _tile_cost(np.maximum(m_wo_t[:, None, :], cb[None, :, :]))
        new_u_sw = _tile_cost(np.maximum(m_wo_u[None, :, :], ct[:, None, :]))
        u_sw = np.repeat(sw[others], P)
        # sum-of-squares on cost: equalizes tiles without inflating the total
        score = (new_t_sw**2 + new_u_sw**2) - (sw[t] ** 2 + u_sw[None, :] ** 2)
        i, j = np.unravel_index(int(score.argmin()), score.shape)
        if score[i, j] >= -1e-9:
            break
        u_idx = others[j // P]
        slot = j % P
        tiles[t][i], tiles[u_idx][slot] = tiles[u_idx][slot], tiles[t][i]
        m[t] = tile_m(t)
        m[u_idx] = tile_m(u_idx)
    return tiles, m


def _cluster(cnt):
    """Greedy-pack 1600 bags into 13 tiles of 128 minimizing sum of per-tile
    per-chunk maxima; best of a few orderings. Returns tiles [N_TILES, P] of
    bag ids (-1 = dummy)."""
    orders = [
        np.argsort(-cnt.max(axis=1), kind="stable"),
        np.argsort(-cnt[:, :3].max(axis=1), kind="stable"),
        np.lexsort((cnt[:, 2], cnt[:, 1], cnt[:, 0]))[::-1],
    ]
    rng = np.random.default_rng(0)
    base = np.argsort(-cnt.max(axis=1), kind="stable")
    for _ in range(12):
        # perturbed difficulty order: keeps hard bags early but varies packing
        noise = rng.normal(0, 1.5, size=len(cnt))
        orders.append(np.argsort(-(cnt.max(axis=1) + noise), kind="stable"))
    for _ in range(3):
        orders.append(rng.permutation(len(cnt)))
    best = None
    for order in orders:
        tiles, m = _cluster_once(cnt, order)
        tot = float(_tile_cost(m).sum())
        if best is None or tot < best[0]:
            best = (tot, tiles, m)
    _, tiles, m = best
    # -1 dummies index the appended all-zeros row of cnt_ext
    cnt_ext = np.vstack([cnt, np.zeros((1, N_CHUNKS), cnt.dtype)])
    tiles, m = _refine(tiles, cnt_ext)
    tiles, m = _refine_balance(tiles, cnt_ext)
    # pyramid order (light tiles at both ends): a light head fills the
    # gather/reduce pipeline fast, a light tail drains the DMA backlog fast
    asc = np.argsort(_tile_cost(m), kind="stable")
    order_t = np.concatenate([asc[0::2], asc[1::2][::-1]])
    return tiles[order_t], m[order_t]


def _pieces(w):
    """Split a chunk width into gather-call pieces so no single SWDGE queue
    carries more than ~15 columns of desc-gen for one tile."""
    if w <= 15:
        return [w]
    h = (w + 1) // 2
    return [h, w - h]


def _wrap_idxs(arr, w):
    """arr [P, w] int16 (partition-major slot grid) -> [128, w*8] wrapped+replicated."""
    L = P * w
    i = np.arange(L)
    lin = arr[i % P, i // P]                     # list position i = col*128 + p
    wrapped = lin.reshape(w * 8, 16).T           # [16, w*8]
    return np.tile(wrapped, (8, 1)).astype(np.int16)


def _prep_inputs(content_input, W, b):
    """Returns (in_maps, widths) where widths[t][c] is shared across cores."""
    ids = np.asarray(content_input).astype(np.int64).reshape(B, S, K)
    Wf = np.asarray(W, dtype=np.float32)
    bf = np.asarray(b, dtype=np.float32)

    ids = np.where(ids == 0, ZERO_V, ids)
    all_bags = ids.reshape(B * S, K)
    dup_ids, dup_rank = _pick_dup_ids(all_bags)
    tabs = _build_tables(Wf, bf, dup_ids)
    per_core = []
    tiles_per_core = []
    widths = np.zeros((N_TILES, N_CHUNKS), np.int64)
    for c in range(N_CORES):
        bag_ids = ids[c * B_PER_CORE : (c + 1) * B_PER_CORE].reshape(BAGS, K)
        plan = _plan_core(bag_ids, dup_rank)  # bias added on-device
        per_core.append(plan)
        tiles, m = _cluster(plan[1])
        tiles_per_core.append(tiles)
        widths = np.maximum(widths, m)

    bias_rep = np.ascontiguousarray(np.tile(bf[None, :], (P, 1)).astype(np.float32))
    in_maps = []
    for core in range(N_CORES):
        IDX_sorted, cnt = per_core[core]
        tiles = tiles_per_core[core]
        planes = []
        for t in range(N_TILES):
            bags = tiles[t]  # [P] bag ids, -1 = dummy
            real = bags >= 0
            bsafe = np.where(real, bags, 0)
            csum = np.zeros((P,), np.int64)
            for ch in range(N_CHUNKS):
                w = int(widths[t][ch])
                if w == 0:
                    continue
                cn = np.where(real, cnt[bsafe, ch], 0)
                j = np.arange(w)[None, :]
                src = np.take_along_axis(
                    IDX_sorted[bsafe], np.minimum(csum[:, None] + j, K - 1), axis=1
                )
                arr = np.where((j < cn[:, None]) & real[:, None], src, 0).astype(
                    np.int16
                )
                a = 0
                for pw in _pieces(w):
                    planes.append(_wrap_idxs(arr[:, a : a + pw], pw))
                    a += pw
                csum += cn
        ids16 = np.concatenate(planes, axis=1)
        m = {"ids16": np.ascontiguousarray(ids16), "bias": bias_rep}
        for c in range(N_CHUNKS):
            m[f"wt{c}"] = tabs[c]
        in_maps.append(m)
    return in_maps, widths, tiles_per_core


def _build_program(widths, ids_cols):
    import concourse.bass as bass
    import concourse.mybir as mybir
    from concourse import bacc
    from concourse.tile import TileContext

    sw = [int(widths[t].sum()) for t in range(N_TILES)]
    sw_max = max(sw)
    # g tile is sw_max*256B per partition; keep total pool under ~160KB/partition
    g_bufs = max(1, min(6, (160 * 1024) // (sw_max * E * 2)))

    nc = bacc.Bacc("TRN2", target_bir_lowering=False, debug=False,
                   num_devices=N_CORES, num_swdge_queues=4)
    ids_d = nc.declare_dram_parameter("ids16", [P, ids_cols], mybir.dt.int16, isOutput=False)
    wt_ds = [
        nc.declare_dram_parameter(
            f"wt{c}", [CHUNK_ROWS[c], E], mybir.dt.bfloat16, isOutput=False
        )
        for c in range(N_CHUNKS)
    ]
    bias_d = nc.declare_dram_parameter("bias", [P, E], mybir.dt.float32, isOutput=False)
    out_d = nc.declare_dram_parameter("out", [BAGS_PAD, E], mybir.dt.float32, isOutput=True)

    with TileContext(nc) as tc:
        with (
            tc.tile_pool(name="ids", bufs=1) as ids_pool,
            tc.tile_pool(name="bias", bufs=1) as bias_pool,
            tc.tile_pool(name="g", bufs=g_bufs) as g_pool,
            tc.tile_pool(name="o", bufs=3) as o_pool,
        ):
            bias_sb = bias_pool.tile([P, E], mybir.dt.float32)
            nc.sync.dma_start(out=bias_sb[:], in_=bias_d[:])
            # split the ids load so tile 0's gathers only wait for its slice
            t0_cols = int(widths[0].sum()) * 8
            ids_sb = ids_pool.tile([P, t0_cols], mybir.dt.int16)
            nc.sync.dma_start(out=ids_sb[:], in_=ids_d[:, :t0_cols])
            ids_sb2 = ids_pool.tile([P, ids_cols - t0_cols], mybir.dt.int16)
            nc.sync.dma_start(out=ids_sb2[:], in_=ids_d[:, t0_cols:])
            off = 0  # free-dim offset into ids16, in idx elements
            qi = 0   # emitted-gather counter; queue = qi % 4 keeps each of
            # Tile's 8 round-robin DMASW lanes locked to a single SWDGE queue
            for t in range(N_TILES):
                g = g_pool.tile([P, sw_max * E], mybir.dt.bfloat16)
                col = 0
                for ch in range(N_CHUNKS):
                    w = int(widths[t][ch])
                    if w == 0:
                        continue
                    for pw in _pieces(w):
                        n = pw * P
                        dst = g[:, col * E : (col + pw) * E].rearrange(
                            "p (j e) -> p j e", j=pw, e=E
                        )
                        if off < t0_cols:
                            idx_ap = ids_sb[:, off : off + pw * 8]
                        else:
                            idx_ap = ids_sb2[:, off - t0_cols : off - t0_cols + pw * 8]
                        nc.gpsimd.dma_gather(
                            dst,
                            wt_ds[ch][:],
                            idx_ap,
                            n,
                            n,
                            E,
                            single_packet=False,
                            queue_num=qi % 4,
                        )
                        qi += 1
                        col += pw
                        off += pw * 8
                o = o_pool.tile([P, E], mybir.dt.float32)
                nc.vector.tensor_reduce(
                    out=o[:],
                    in_=g[:, : sw[t] * E].rearrange("p (j e) -> p e j", j=sw[t], e=E),
                    axis=mybir.AxisListType.X,
                    op=mybir.AluOpType.add,
                )
                nc.vector.tensor_add(out=o[:], in0=o[:], in1=bias_sb[:])
                nc.sync.dma_start(out=out_d[t * P : (t + 1) * P, :], in_=o[:])
    nc.compile()
    return nc


def kernel(content_input, W, b):
    global LAST_RESULTS
    in_maps, widths, tiles_per_core = _prep_inputs(content_input, W, b)
    ids_cols = in_maps[0]["ids16"].shape[1]
    nc = _build_program(widths, ids_cols)

    from concourse.bass_utils import run_bass_kernel_spmd

    res = run_bass_kernel_spmd(nc, in_maps, list(range(N_CORES)), trace=TRACE)
    LAST_RESULTS = {
        "exec_time_ns": res.exec_time_ns,
        "mean_exec_time_ns": res.mean_exec_time_ns,
        "instructions_and_trace": res.instructions_and_trace,
        "profile_json": res.profile_json,
        "widths": widths,
    }

    out = np.empty((B, S, E), np.float32)
    for c in range(N_CORES):
        rows = res.results[c]["out"]  # [BAGS_PAD, E], permuted bag order
        flat = np.empty((BAGS, E), np.float32)
        tiles = tiles_per_core[c].reshape(BAGS_PAD)
        real = tiles >= 0
        flat[tiles[real]] = rows[real]
        out[c * B_PER_CORE : (c + 1) * B_PER_CORE] = flat.reshape(B_PER_CORE, S, E)
    return out



# revision 26
# speedup vs baseline: 2.8733x; 1.0083x over previous
"""Trainium2 Bass kernel for nn_LinearUpscaler (masked embedding-bag sum + bias).

reference:  g = W.T[ids]; g[ids == 0] = 0; out = g.sum(axis=2) + b

Design: data-parallel over batch across 8 cores (8 batch rows -> 1600 bags of
51 slots each: 50 items + 1 bias-row slot; ids==0 remapped to a zero row).

The gather engine is the GPSIMD dma_gather custom op (int16 indices, one
512B/256B row per index, written to partition i%128, column i//128).  Since
indices are signed int16 (max 32767) and V=100002, the fp16 table is split
into 4 vocab chunks; row 0 of each chunk is a zero row so padding slots can
gather harmlessly.  For each tile of 128 bags, each bag's slots are bucketed
by chunk; per (tile, chunk) all bags are padded to a common width W so the
dma_gather index list is fully valid (no negative indices) with a
compile-time num_idxs.  The list order is chosen so bag b's rows land in
partition b at consecutive columns; one strided vector-engine reduce per tile
sums items+chunks+padding (pads contribute zeros) in a single pass, and the
result is DMA'd out.  The program is specialized per call (widths depend on
the actual ids); no collectives are needed.
"""

import importlib.util
import os
import sys

if importlib.util.find_spec("concourse") is None:
    for _p in ("/opt/trn_rl_repo", "/root/.axon_site/_ro/trn_rl_repo"):
        if os.path.isdir(_p) and _p not in sys.path:
            sys.path.insert(0, _p)
            break

import numpy as np

try:
    import ml_dtypes
    _BF16 = ml_dtypes.bfloat16
except ImportError:
    _BF16 = None


def _to_bf16(a):
    """f32 -> bf16 (round-to-nearest-even); uint16 view fallback."""
    if _BF16 is not None:
        return a.astype(_BF16)
    x = np.ascontiguousarray(a, np.float32).view(np.uint32)
    r = ((x >> 16) & 1) + 0x7FFF
    return ((x + r) >> 16).astype(np.uint16)


N_CORES = 8
B, S, K = 64, 200, 50
V, E = 100000, 128
KE = K + 1            # items + bias slot
BIAS_V = V            # logical row V   = b
ZERO_V = V + 1        # logical row V+1 = 0
NV = V + 2            # logical vocab incl. bias+zero rows
P = 128
B_PER_CORE = B // N_CORES
BAGS = B_PER_CORE * S               # 1600 bags per core
N_TILES = -(-BAGS // P)             # 13
BAGS_PAD = N_TILES * P              # 1664

CHUNK_CAP = 32767                   # real rows per chunk (idx 1..32767)
N_CHUNKS = -(-NV // CHUNK_CAP)      # 4
# chunk 3 is mostly empty (1701 natural rows); fill its spare idx space with
# DUPLICATES of chunk-0..2 rows (chosen per call, by usefulness for shedding
# over-full bags) so bags can rebalance slots across chunks, smoothing the
# per-chunk counts that drive padding
N_NAT3 = NV - 3 * CHUNK_CAP         # 1701 natural chunk-3 rows
DUP_BASE = N_NAT3 + 1               # chunk-3 idx of first duplicate
N_DUP = CHUNK_CAP - DUP_BASE + 1    # 31066 duplicate slots
CHUNK_ROWS = [CHUNK_CAP + 1] * 3 + [DUP_BASE + N_DUP]


def _pick_dup_ids(all_bags, level=13, extra_score=None):
    """Choose the N_DUP most useful rows to duplicate into chunk 3: score each
    id by its occurrences inside (bag, chunk) pairs that are over-full."""
    C = all_bags // CHUNK_CAP
    cnt = np.stack([(C == c).sum(axis=1) for c in range(N_CHUNKS)], axis=1)
    need = cnt > level                      # [bags, NC]
    useful = need[np.arange(len(all_bags))[:, None], C] & (all_bags < 3 * CHUNK_CAP)
    score = np.bincount(
        all_bags[useful].astype(np.int64), minlength=3 * CHUNK_CAP
    ).astype(np.float64)
    if extra_score is not None:
        score += extra_score
    dup_ids = np.sort(np.argsort(-score, kind="stable")[:N_DUP])
    dup_rank = np.full(3 * CHUNK_CAP, -1, np.int64)
    dup_rank[dup_ids] = np.arange(N_DUP)
    return dup_ids, dup_rank

TRACE = False       # test.py flips this to profile
LAST_RESULTS = {}   # test.py reads exec_time_ns etc. from here


def _build_tables(W, b, dup_ids):
    """bf16 chunk tables, each [zero row; <=CHUNK_CAP vocab rows].

    Separate tensors (not slices of one): the gather ucode's row addressing
    breaks when AP-base-offset + idx exceeds 32767 rows."""
    wt = np.zeros((NV, E), np.float32)
    wt[:V] = W.T
    wt[BIAS_V] = b
    tabs = []
    for c in range(3):
        t = np.zeros((CHUNK_ROWS[c], E), np.float32)
        t[1:] = wt[CHUNK_CAP * c : CHUNK_CAP * (c + 1)]
        tabs.append(_to_bf16(t))
    t3 = np.zeros((CHUNK_ROWS[3], E), np.float32)
    t3[1 : 1 + N_NAT3] = wt[3 * CHUNK_CAP :]
    t3[DUP_BASE:] = wt[dup_ids]
    tabs.append(_to_bf16(t3))
    return tabs


def _plan_core(v_bags, dup_rank):
    """v_bags: [BAGS, K] logical rows. Returns per-bag chunk-sorted idx lists
    and per-chunk counts, after rebalancing duplicate-eligible slots from
    over-full chunks 0..2 into chunk 3 to minimize each bag's max count.

    sorted_idx[bag, j] = local int16 idx of the bag's j-th slot when slots are
    ordered by (rebalanced) chunk; cnt[bag, c] = slots in chunk c."""
    C = v_bags // CHUNK_CAP                      # [BAGS, K] natural chunk
    elig = (v_bags < 3 * CHUNK_CAP) & (
        dup_rank[np.minimum(v_bags, 3 * CHUNK_CAP - 1)] >= 0
    )
    C2 = C.copy()
    for b in range(v_bags.shape[0]):
        row = C2[b]
        c = [(row == x).sum() for x in range(N_CHUNKS)]
        movable = [list(np.where((row == x) & elig[b])[0]) for x in range(3)]
        while True:
            moved = False
            for mx in sorted(range(3), key=lambda x: -c[x]):
                if c[mx] <= c[3] + 1:
                    break
                if movable[mx]:
                    row[movable[mx].pop()] = 3
                    c[mx] -= 1
                    c[3] += 1
                    moved = True
                    break
            if not moved:
                break
    IDX = np.where(
        C2 == C,
        v_bags - C * CHUNK_CAP + 1,
        DUP_BASE + dup_rank[np.minimum(v_bags, 3 * CHUNK_CAP - 1)],
    ).astype(np.int16)
    order = np.argsort(C2, axis=1, kind="stable")  # chunk-major slot order
    IDX_sorted = np.take_along_axis(IDX, order, axis=1)
    cnt = np.stack([(C2 == c).sum(axis=1) for c in range(N_CHUNKS)], axis=1)
    return IDX_sorted, cnt


# Per-tile pipeline cost (width units): desc-gen waits on the widest chunk's
# queue (128*7.57ns per width unit => 3.73x), the DVE reduce on the chunk-width
# sum (128*2.03ns per unit => 1x); the tile costs whichever engine is slower.
_COST_MAXC = 3.73


def _tile_cost(m):
    """m: [..., NC] per-chunk widths -> per-tile pipeline cost."""
    return np.maximum(_COST_MAXC * m.max(axis=-1), m.sum(axis=-1))


def _cluster_once(cnt, order):
    m = np.zeros((N_TILES, N_CHUNKS), np.int64)
    fill = np.zeros(N_TILES, np.int64)
    tiles = np.full((N_TILES, P), -1, np.int64)
    for b in order:
        best_key, best_t = None, None
        for t in range(N_TILES):
            if fill[t] >= P:
                continue
            nm = np.maximum(m[t], cnt[b])
            inc = float(_tile_cost(nm) - _tile_cost(m[t]))
            key = (inc, -int(fill[t]))
            if best_key is None or key < best_key:
                best_key, best_t = key, t
        tiles[best_t, fill[best_t]] = b
        m[best_t] = np.maximum(m[best_t], cnt[b])
        fill[best_t] += 1
    return tiles, m


def _refine(tiles, cnt_ext, iters=120):
    """Swap-based local search: repeatedly swap a bag out of the widest tile
    when it lowers the summed per-tile per-chunk maxima."""

    def tile_m(t):
        return cnt_ext[tiles[t]].max(axis=0)

    def max_without(members):
        """[P, NC] per-chunk max over members excluding each member."""
        ct = cnt_ext[members]
        srt = np.sort(ct, axis=0)
        top1, top2 = srt[-1], srt[-2]
        is_top = ct == top1[None, :]
        uniq = is_top.sum(axis=0) == 1
        return ct, np.where(is_top & uniq[None, :], top2[None, :], top1[None, :])

    m = np.stack([tile_m(t) for t in range(N_TILES)])
    for _ in range(iters):
        t = int(_tile_cost(m).argmax())
        ct, m_wo_t = max_without(tiles[t])
        others = [u for u in range(N_TILES) if u != t]
        cb_list, m_wo_list = zip(*(max_without(tiles[u]) for u in others))
        cb = np.concatenate(cb_list)           # [M, NC] candidate counts
        m_wo_u = np.concatenate(m_wo_list)     # [M, NC] u's width w/o candidate
        # widths of t after swapping member i with candidate j
        new_t = np.maximum(m_wo_t[:, None, :], cb[None, :, :])  # [P, M, NC]
        d_t = _tile_cost(new_t) - _tile_cost(m[t])
        # exact widths of u after losing candidate j and receiving member i
        new_u = np.maximum(m_wo_u[None, :, :], ct[:, None, :])  # [P, M, NC]
        u_costs = np.repeat(_tile_cost(m[others]), P)
        d_u = _tile_cost(new_u) - u_costs[None, :]
        delta = d_t + d_u
        i, j = np.unravel_index(int(delta.argmin()), delta.shape)
        if delta[i, j] >= -1e-9:
            break
        u_idx = others[j // P]  # j indexes (tile-in-others, slot)
        slot = j % P
        tiles[t][i], tiles[u_idx][slot] = tiles[u_idx][slot], tiles[t][i]
        m[t] = tile_m(t)
        m[u_idx] = tile_m(u_idx)
    return tiles, m


def _refine_balance(tiles, cnt_ext, iters=400):
    """Second local-search phase: minimize (max per-tile total width, sum),
    so the 13 tiles pipeline evenly (no heavy head-of-line tile)."""

    def tile_m(t):
        return cnt_ext[tiles[t]].max(axis=0)

    def max_without(members):
        ct = cnt_ext[members]
        srt = np.sort(ct, axis=0)
        top1, top2 = srt[-1], srt[-2]
        is_top = ct == top1[None, :]
        uniq = is_top.sum(axis=0) == 1
        return ct, np.where(is_top & uniq[None, :], top2[None, :], top1[None, :])

    m = np.stack([tile_m(t) for t in range(N_TILES)])
    for _ in range(iters):
        sw = _tile_cost(m)
        t = int(sw.argmax())
        ct, m_wo_t = max_without(tiles[t])
        others = [u for u in range(N_TILES) if u != t]
        cb_list, m_wo_list = zip(*(max_without(tiles[u]) for u in others))
        cb = np.concatenate(cb_list)
        m_wo_u = np.concatenate(m_wo_list)
        new_t_sw = _tile_cost(np.maximum(m_wo_t[:, None, :], cb[None, :, :]))
        new_u_sw = _tile_cost(np.maximum(m_wo_u[None, :, :], ct[:, None, :]))
        u_sw = np.repeat(sw[others], P)
        # sum-of-squares on cost: equalizes tiles without inflating the total
        score = (new_t_sw**2 + new_u_sw**2) - (sw[t] ** 2 + u_sw[None, :] ** 2)
        i, j = np.unravel_index(int(score.argmin()), score.shape)
        if score[i, j] >= -1e-9:
            break
        u_idx = others[j // P]
        slot = j % P
        tiles[t][i], tiles[u_idx][slot] = tiles[u_idx][slot], tiles[t][i]
        m[t] = tile_m(t)
        m[u_idx] = tile_m(u_idx)
    return tiles, m


def _cluster(cnt):
    """Greedy-pack 1600 bags into 13 tiles of 128 minimizing sum of per-tile
    per-chunk maxima; best of a few orderings. Returns tiles [N_TILES, P] of
    bag ids (-1 = dummy)."""
    orders = [
        np.argsort(-cnt.max(axis=1), kind="stable"),
        np.argsort(-cnt[:, :3].max(axis=1), kind="stable"),
        np.lexsort((cnt[:, 2], cnt[:, 1], cnt[:, 0]))[::-1],
    ]
    rng = np.random.default_rng(0)
    base = np.argsort(-cnt.max(axis=1), kind="stable")
    for _ in range(12):
        # perturbed difficulty order: keeps hard bags early but varies packing
        noise = rng.normal(0, 1.5, size=len(cnt))
        orders.append(np.argsort(-(cnt.max(axis=1) + noise), kind="stable"))
    for _ in range(3):
        orders.append(rng.permutation(len(cnt)))
    best = None
    for order in orders:
        tiles, m = _cluster_once(cnt, order)
        tot = float(_tile_cost(m).sum())
        if best is None or tot < best[0]:
            best = (tot, tiles, m)
    _, tiles, m = best
    # -1 dummies index the appended all-zeros row of cnt_ext
    cnt_ext = np.vstack([cnt, np.zeros((1, N_CHUNKS), cnt.dtype)])
    tiles, m = _refine(tiles, cnt_ext)
    tiles, m = _refine_balance(tiles, cnt_ext)
    # pyramid order (light tiles at both ends): a light head fills the
    # gather/reduce pipeline fast, a light tail drains the DMA backlog fast
    asc = np.argsort(_tile_cost(m), kind="stable")
    order_t = np.concatenate([asc[0::2], asc[1::2][::-1]])
    return tiles[order_t], m[order_t]


def _pieces(w):
    """Split a chunk width into gather-call pieces so no single SWDGE queue
    carries more than ~15 columns of desc-gen for one tile."""
    if w <= 15:
        return [w]
    h = (w + 1) // 2
    return [h, w - h]


def _wrap_idxs(arr, w):
    """arr [P, w] int16 (partition-major slot grid) -> [128, w*8] wrapped+replicated."""
    L = P * w
    i = np.arange(L)
    lin = arr[i % P, i // P]                     # list position i = col*128 + p
    wrapped = lin.reshape(w * 8, 16).T           # [16, w*8]
    return np.tile(wrapped, (8, 1)).astype(np.int16)


def _prep_inputs(content_input, W, b):
    """Returns (in_maps, widths) where widths[t][c] is shared across cores."""
    ids = np.asarray(content_input).astype(np.int64).reshape(B, S, K)
    Wf = np.asarray(W, dtype=np.float32)
    bf = np.asarray(b, dtype=np.float32)

    ids = np.where(ids == 0, ZERO_V, ids)
    all_bags = ids.reshape(B * S, K)
    dup_ids, dup_rank = _pick_dup_ids(all_bags)
    tabs = _build_tables(Wf, bf, dup_ids)
    per_core = []
    tiles_per_core = []
    widths = np.zeros((N_TILES, N_CHUNKS), np.int64)
    for c in range(N_CORES):
        bag_ids = ids[c * B_PER_CORE : (c + 1) * B_PER_CORE].reshape(BAGS, K)
        plan = _plan_core(bag_ids, dup_rank)  # bias added on-device
        per_core.append(plan)
        tiles, m = _cluster(plan[1])
        tiles_per_core.append(tiles)
        widths = np.maximum(widths, m)

    bias_rep = np.ascontiguousarray(np.tile(bf[None, :], (P, 1)).astype(np.float32))
    in_maps = []
    for core in range(N_CORES):
        IDX_sorted, cnt = per_core[core]
        tiles = tiles_per_core[core]
        planes = []
        for t in range(N_TILES):
            bags = tiles[t]  # [P] bag ids, -1 = dummy
            real = bags >= 0
            bsafe = np.where(real, bags, 0)
            csum = np.zeros((P,), np.int64)
            for ch in range(N_CHUNKS):
                w = int(widths[t][ch])
                if w == 0:
                    continue
                cn = np.where(real, cnt[bsafe, ch], 0)
                j = np.arange(w)[None, :]
                src = np.take_along_axis(
                    IDX_sorted[bsafe], np.minimum(csum[:, None] + j, K - 1), axis=1
                )
                arr = np.where((j < cn[:, None]) & real[:, None], src, 0).astype(
                    np.int16
                )
                a = 0
                for pw in _pieces(w):
                    planes.append(_wrap_idxs(arr[:, a : a + pw], pw))
                    a += pw
                csum += cn
        ids16 = np.concatenate(planes, axis=1)
        m = {"ids16": np.ascontiguousarray(ids16), "bias": bias_rep}
        for c in range(N_CHUNKS):
            m[f"wt{c}"] = tabs[c]
        in_maps.append(m)
    return in_maps, widths, tiles_per_core


def _build_program(widths, ids_cols):
    import concourse.bass as bass
    import concourse.mybir as mybir
    from concourse import bacc
    from concourse.tile import TileContext

    sw = [int(widths[t].sum()) for t in range(N_TILES)]
    sw_max = max(sw)
    # g tile is sw_max*256B per partition; keep total pool under ~160KB/partition
    g_bufs = max(1, min(8, (120 * 1024) // (sw_max * E * 2)))

    nc = bacc.Bacc("TRN2", target_bir_lowering=False, debug=False,
                   num_devices=N_CORES, num_swdge_queues=4,
                   dynamic_dma_scratch_size=32768)
    ids_d = nc.declare_dram_parameter("ids16", [P, ids_cols], mybir.dt.int16, isOutput=False)
    wt_ds = [
        nc.declare_dram_parameter(
            f"wt{c}", [CHUNK_ROWS[c], E], mybir.dt.bfloat16, isOutput=False
        )
        for c in range(N_CHUNKS)
    ]
    bias_d = nc.declare_dram_parameter("bias", [P, E], mybir.dt.float32, isOutput=False)
    out_d = nc.declare_dram_parameter("out", [BAGS_PAD, E], mybir.dt.float32, isOutput=True)

    with TileContext(nc) as tc:
        with (
            tc.tile_pool(name="ids", bufs=1) as ids_pool,
            tc.tile_pool(name="bias", bufs=1) as bias_pool,
            tc.tile_pool(name="g", bufs=g_bufs) as g_pool,
            tc.tile_pool(name="o", bufs=3) as o_pool,
        ):
            bias_sb = bias_pool.tile([P, E], mybir.dt.float32)
            nc.sync.dma_start(out=bias_sb[:], in_=bias_d[:])
            # split the ids load so tile 0's gathers only wait for its slice
            t0_cols = int(widths[0].sum()) * 8
            ids_sb = ids_pool.tile([P, t0_cols], mybir.dt.int16)
            nc.sync.dma_start(out=ids_sb[:], in_=ids_d[:, :t0_cols])
            ids_sb2 = ids_pool.tile([P, ids_cols - t0_cols], mybir.dt.int16)
            nc.sync.dma_start(out=ids_sb2[:], in_=ids_d[:, t0_cols:])
            off = 0  # free-dim offset into ids16, in idx elements
            qi = 0   # emitted-gather counter; queue = qi % 4 keeps each of
            # Tile's 8 round-robin DMASW lanes locked to a single SWDGE queue
            for t in range(N_TILES):
                g = g_pool.tile([P, sw_max * E], mybir.dt.bfloat16)
                col = 0
                for ch in range(N_CHUNKS):
                    w = int(widths[t][ch])
                    if w == 0:
                        continue
                    for pw in _pieces(w):
                        n = pw * P
                        dst = g[:, col * E : (col + pw) * E].rearrange(
                            "p (j e) -> p j e", j=pw, e=E
                        )
                        if off < t0_cols:
                            idx_ap = ids_sb[:, off : off + pw * 8]
                        else:
                            idx_ap = ids_sb2[:, off - t0_cols : off - t0_cols + pw * 8]
                        nc.gpsimd.dma_gather(
                            dst,
                            wt_ds[ch][:],
                            idx_ap,
                            n,
                            n,
                            E,
                            single_packet=False,
                            queue_num=qi % 4,
                        )
                        qi += 1
                        col += pw
                        off += pw * 8
                o = o_pool.tile([P, E], mybir.dt.float32)
                nc.vector.tensor_reduce(
                    out=o[:],
                    in_=g[:, : sw[t] * E].rearrange("p (j e) -> p e j", j=sw[t], e=E),
                    axis=mybir.AxisListType.X,
                    op=mybir.AluOpType.add,
                )
                nc.vector.tensor_add(out=o[:], in0=o[:], in1=bias_sb[:])
                nc.sync.dma_start(out=out_d[t * P : (t + 1) * P, :], in_=o[:])
    nc.compile()
    return nc


def kernel(content_input, W, b):
    global LAST_RESULTS
    in_maps, widths, tiles_per_core = _prep_inputs(content_input, W, b)
    ids_cols = in_maps[0]["ids16"].shape[1]
    nc = _build_program(widths, ids_cols)

    from concourse.bass_utils import run_bass_kernel_spmd

    res = run_bass_kernel_spmd(nc, in_maps, list(range(N_CORES)), trace=TRACE)
    LAST_RESULTS = {
        "exec_time_ns": res.exec_time_ns,
        "mean_exec_time_ns": res.mean_exec_time_ns,
        "instructions_and_trace": res.instructions_and_trace,
        "profile_json": res.profile_json,
        "widths": widths,
    }

    out = np.empty((B, S, E), np.float32)
    for c in range(N_CORES):
        rows = res.results[c]["out"]  # [BAGS_PAD, E], permuted bag order
        flat = np.empty((BAGS, E), np.float32)
        tiles = tiles_per_core[c].reshape(BAGS_PAD)
        real = tiles >= 0
        flat[tiles[real]] = rows[real]
        out[c * B_PER_CORE : (c + 1) * B_PER_CORE] = flat.reshape(B_PER_CORE, S, E)
    return out



# revision 27
# speedup vs baseline: 2.9019x; 1.0099x over previous
"""Trainium2 Bass kernel for nn_LinearUpscaler (masked embedding-bag sum + bias).

reference:  g = W.T[ids]; g[ids == 0] = 0; out = g.sum(axis=2) + b

Design: data-parallel over batch across 8 cores (8 batch rows -> 1600 bags of
51 slots each: 50 items + 1 bias-row slot; ids==0 remapped to a zero row).

The gather engine is the GPSIMD dma_gather custom op (int16 indices, one
512B/256B row per index, written to partition i%128, column i//128).  Since
indices are signed int16 (max 32767) and V=100002, the fp16 table is split
into 4 vocab chunks; row 0 of each chunk is a zero row so padding slots can
gather harmlessly.  For each tile of 128 bags, each bag's slots are bucketed
by chunk; per (tile, chunk) all bags are padded to a common width W so the
dma_gather index list is fully valid (no negative indices) with a
compile-time num_idxs.  The list order is chosen so bag b's rows land in
partition b at consecutive columns; one strided vector-engine reduce per tile
sums items+chunks+padding (pads contribute zeros) in a single pass, and the
result is DMA'd out.  The program is specialized per call (widths depend on
the actual ids); no collectives are needed.
"""

import importlib.util
import os
import sys

if importlib.util.find_spec("concourse") is None:
    for _p in ("/opt/trn_rl_repo", "/root/.axon_site/_ro/trn_rl_repo"):
        if os.path.isdir(_p) and _p not in sys.path:
            sys.path.insert(0, _p)
            break

import numpy as np

try:
    import ml_dtypes
    _BF16 = ml_dtypes.bfloat16
except ImportError:
    _BF16 = None


def _to_bf16(a):
    """f32 -> bf16 (round-to-nearest-even); uint16 view fallback."""
    if _BF16 is not None:
        return a.astype(_BF16)
    x = np.ascontiguousarray(a, np.float32).view(np.uint32)
    r = ((x >> 16) & 1) + 0x7FFF
    return ((x + r) >> 16).astype(np.uint16)


N_CORES = 8
B, S, K = 64, 200, 50
V, E = 100000, 128
KE = K + 1            # items + bias slot
BIAS_V = V            # logical row V   = b
ZERO_V = V + 1        # logical row V+1 = 0
NV = V + 2            # logical vocab incl. bias+zero rows
P = 128
B_PER_CORE = B // N_CORES
BAGS = B_PER_CORE * S               # 1600 bags per core
N_TILES = -(-BAGS // P)             # 13
BAGS_PAD = N_TILES * P              # 1664

CHUNK_CAP = 32767                   # real rows per chunk (idx 1..32767)
N_CHUNKS = -(-NV // CHUNK_CAP)      # 4
# chunk 3 is mostly empty (1701 natural rows); fill its spare idx space with
# DUPLICATES of chunk-0..2 rows (chosen per call, by usefulness for shedding
# over-full bags) so bags can rebalance slots across chunks, smoothing the
# per-chunk counts that drive padding
N_NAT3 = NV - 3 * CHUNK_CAP         # 1701 natural chunk-3 rows
DUP_BASE = N_NAT3 + 1               # chunk-3 idx of first duplicate
N_DUP = CHUNK_CAP - DUP_BASE + 1    # 31066 duplicate slots
CHUNK_ROWS = [CHUNK_CAP + 1] * 3 + [DUP_BASE + N_DUP]


def _pick_dup_ids(all_bags, level=13, extra_score=None):
    """Choose the N_DUP most useful rows to duplicate into chunk 3: score each
    id by its occurrences inside (bag, chunk) pairs that are over-full."""
    C = all_bags // CHUNK_CAP
    cnt = np.stack([(C == c).sum(axis=1) for c in range(N_CHUNKS)], axis=1)
    need = cnt > level                      # [bags, NC]
    useful = need[np.arange(len(all_bags))[:, None], C] & (all_bags < 3 * CHUNK_CAP)
    score = np.bincount(
        all_bags[useful].astype(np.int64), minlength=3 * CHUNK_CAP
    ).astype(np.float64)
    if extra_score is not None:
        score += extra_score
    dup_ids = np.sort(np.argsort(-score, kind="stable")[:N_DUP])
    dup_rank = np.full(3 * CHUNK_CAP, -1, np.int64)
    dup_rank[dup_ids] = np.arange(N_DUP)
    return dup_ids, dup_rank

TRACE = False       # test.py flips this to profile
LAST_RESULTS = {}   # test.py reads exec_time_ns etc. from here


def _build_tables(W, b, dup_ids):
    """bf16 chunk tables, each [zero row; <=CHUNK_CAP vocab rows].

    Separate tensors (not slices of one): the gather ucode's row addressing
    breaks when AP-base-offset + idx exceeds 32767 rows."""
    wt = np.zeros((NV, E), np.float32)
    wt[:V] = W.T
    wt[BIAS_V] = b
    tabs = []
    for c in range(3):
        t = np.zeros((CHUNK_ROWS[c], E), np.float32)
        t[1:] = wt[CHUNK_CAP * c : CHUNK_CAP * (c + 1)]
        tabs.append(_to_bf16(t))
    t3 = np.zeros((CHUNK_ROWS[3], E), np.float32)
    t3[1 : 1 + N_NAT3] = wt[3 * CHUNK_CAP :]
    t3[DUP_BASE:] = wt[dup_ids]
    tabs.append(_to_bf16(t3))
    return tabs


def _plan_core(v_bags, dup_rank):
    """v_bags: [BAGS, K] logical rows. Returns per-bag chunk-sorted idx lists
    and per-chunk counts, after rebalancing duplicate-eligible slots from
    over-full chunks 0..2 into chunk 3 to minimize each bag's max count.

    sorted_idx[bag, j] = local int16 idx of the bag's j-th slot when slots are
    ordered by (rebalanced) chunk; cnt[bag, c] = slots in chunk c."""
    C = v_bags // CHUNK_CAP                      # [BAGS, K] natural chunk
    elig = (v_bags < 3 * CHUNK_CAP) & (
        dup_rank[np.minimum(v_bags, 3 * CHUNK_CAP - 1)] >= 0
    )
    C2 = C.copy()
    for b in range(v_bags.shape[0]):
        row = C2[b]
        c = [(row == x).sum() for x in range(N_CHUNKS)]
        movable = [list(np.where((row == x) & elig[b])[0]) for x in range(3)]
        while True:
            moved = False
            for mx in sorted(range(3), key=lambda x: -c[x]):
                if c[mx] <= c[3] + 1:
                    break
                if movable[mx]:
                    row[movable[mx].pop()] = 3
                    c[mx] -= 1
                    c[3] += 1
                    moved = True
                    break
            if not moved:
                break
    IDX = np.where(
        C2 == C,
        v_bags - C * CHUNK_CAP + 1,
        DUP_BASE + dup_rank[np.minimum(v_bags, 3 * CHUNK_CAP - 1)],
    ).astype(np.int16)
    order = np.argsort(C2, axis=1, kind="stable")  # chunk-major slot order
    IDX_sorted = np.take_along_axis(IDX, order, axis=1)
    cnt = np.stack([(C2 == c).sum(axis=1) for c in range(N_CHUNKS)], axis=1)
    return IDX_sorted, cnt


# Per-tile pipeline cost (width units): desc-gen waits on the widest chunk's
# queue (128*7.57ns per width unit => 3.73x), the DVE reduce on the chunk-width
# sum (128*2.03ns per unit => 1x); the tile costs whichever engine is slower.
_COST_MAXC = 3.73


def _tile_cost(m):
    """m: [..., NC] per-chunk widths -> per-tile pipeline cost."""
    return np.maximum(_COST_MAXC * m.max(axis=-1), m.sum(axis=-1))


def _cluster_once(cnt, order):
    m = np.zeros((N_TILES, N_CHUNKS), np.int64)
    fill = np.zeros(N_TILES, np.int64)
    tiles = np.full((N_TILES, P), -1, np.int64)
    for b in order:
        best_key, best_t = None, None
        for t in range(N_TILES):
            if fill[t] >= P:
                continue
            nm = np.maximum(m[t], cnt[b])
            inc = float(_tile_cost(nm) - _tile_cost(m[t]))
            key = (inc, -int(fill[t]))
            if best_key is None or key < best_key:
                best_key, best_t = key, t
        tiles[best_t, fill[best_t]] = b
        m[best_t] = np.maximum(m[best_t], cnt[b])
        fill[best_t] += 1
    return tiles, m


def _refine(tiles, cnt_ext, iters=120):
    """Swap-based local search: repeatedly swap a bag out of the widest tile
    when it lowers the summed per-tile per-chunk maxima."""

    def tile_m(t):
        return cnt_ext[tiles[t]].max(axis=0)

    def max_without(members):
        """[P, NC] per-chunk max over members excluding each member."""
        ct = cnt_ext[members]
        srt = np.sort(ct, axis=0)
        top1, top2 = srt[-1], srt[-2]
        is_top = ct == top1[None, :]
        uniq = is_top.sum(axis=0) == 1
        return ct, np.where(is_top & uniq[None, :], top2[None, :], top1[None, :])

    m = np.stack([tile_m(t) for t in range(N_TILES)])
    for _ in range(iters):
        t = int(_tile_cost(m).argmax())
        ct, m_wo_t = max_without(tiles[t])
        others = [u for u in range(N_TILES) if u != t]
        cb_list, m_wo_list = zip(*(max_without(tiles[u]) for u in others))
        cb = np.concatenate(cb_list)           # [M, NC] candidate counts
        m_wo_u = np.concatenate(m_wo_list)     # [M, NC] u's width w/o candidate
        # widths of t after swapping member i with candidate j
        new_t = np.maximum(m_wo_t[:, None, :], cb[None, :, :])  # [P, M, NC]
        d_t = _tile_cost(new_t) - _tile_cost(m[t])
        # exact widths of u after losing candidate j and receiving member i
        new_u = np.maximum(m_wo_u[None, :, :], ct[:, None, :])  # [P, M, NC]
        u_costs = np.repeat(_tile_cost(m[others]), P)
        d_u = _tile_cost(new_u) - u_costs[None, :]
        delta = d_t + d_u
        i, j = np.unravel_index(int(delta.argmin()), delta.shape)
        if delta[i, j] >= -1e-9:
            break
        u_idx = others[j // P]  # j indexes (tile-in-others, slot)
        slot = j % P
        tiles[t][i], tiles[u_idx][slot] = tiles[u_idx][slot], tiles[t][i]
        m[t] = tile_m(t)
        m[u_idx] = tile_m(u_idx)
    return tiles, m


def _refine_balance(tiles, cnt_ext, iters=400):
    """Second local-search phase: minimize (max per-tile total width, sum),
    so the 13 tiles pipeline evenly (no heavy head-of-line tile)."""

    def tile_m(t):
        return cnt_ext[tiles[t]].max(axis=0)

    def max_without(members):
        ct = cnt_ext[members]
        srt = np.sort(ct, axis=0)
        top1, top2 = srt[-1], srt[-2]
        is_top = ct == top1[None, :]
        uniq = is_top.sum(axis=0) == 1
        return ct, np.where(is_top & uniq[None, :], top2[None, :], top1[None, :])

    m = np.stack([tile_m(t) for t in range(N_TILES)])
    for _ in range(iters):
        sw = _tile_cost(m)
        t = int(sw.argmax())
        ct, m_wo_t = max_without(tiles[t])
        others = [u for u in range(N_TILES) if u != t]
        cb_list, m_wo_list = zip(*(max_without(tiles[u]) for u in others))
        cb = np.concatenate(cb_list)
        m_wo_u = np.concatenate(m_wo_list)
        new_t_sw = _tile_cost(np.maximum(m_wo_t[:, None, :], cb[None, :, :]))
        new_u_sw = _tile_cost(np.maximum(m_wo_u[None, :, :], ct[:, None, :]))
        u_sw = np.repeat(sw[others], P)
        # sum-of-squares on cost: equalizes tiles without inflating the total
        score = (new_t_sw**2 + new_u_sw**2) - (sw[t] ** 2 + u_sw[None, :] ** 2)
        i, j = np.unravel_index(int(score.argmin()), score.shape)
        if score[i, j] >= -1e-9:
            break
        u_idx = others[j // P]
        slot = j % P
        tiles[t][i], tiles[u_idx][slot] = tiles[u_idx][slot], tiles[t][i]
        m[t] = tile_m(t)
        m[u_idx] = tile_m(u_idx)
    return tiles, m


def _cluster(cnt):
    """Greedy-pack 1600 bags into 13 tiles of 128 minimizing sum of per-tile
    per-chunk maxima; best of a few orderings. Returns tiles [N_TILES, P] of
    bag ids (-1 = dummy)."""
    orders = [
        np.argsort(-cnt.max(axis=1), kind="stable"),
        np.argsort(-cnt[:, :3].max(axis=1), kind="stable"),
        np.lexsort((cnt[:, 2], cnt[:, 1], cnt[:, 0]))[::-1],
    ]
    rng = np.random.default_rng(0)
    base = np.argsort(-cnt.max(axis=1), kind="stable")
    for _ in range(12):
        # perturbed difficulty order: keeps hard bags early but varies packing
        noise = rng.normal(0, 1.5, size=len(cnt))
        orders.append(np.argsort(-(cnt.max(axis=1) + noise), kind="stable"))
    for _ in range(3):
        orders.append(rng.permutation(len(cnt)))
    best = None
    for order in orders:
        tiles, m = _cluster_once(cnt, order)
        tot = float(_tile_cost(m).sum())
        if best is None or tot < best[0]:
            best = (tot, tiles, m)
    _, tiles, m = best
    # -1 dummies index the appended all-zeros row of cnt_ext
    cnt_ext = np.vstack([cnt, np.zeros((1, N_CHUNKS), cnt.dtype)])
    tiles, m = _refine(tiles, cnt_ext)
    tiles, m = _refine_balance(tiles, cnt_ext)
    # pyramid order (light tiles at both ends): a light head fills the
    # gather/reduce pipeline fast, a light tail drains the DMA backlog fast
    asc = np.argsort(_tile_cost(m), kind="stable")
    order_t = np.concatenate([asc[0::2], asc[1::2][::-1]])
    return tiles[order_t], m[order_t]


def _pieces(w):
    """Split a chunk width into gather-call pieces so no single SWDGE queue
    carries more than ~15 columns of desc-gen for one tile."""
    if w <= 15:
        return [w]
    h = (w + 1) // 2
    return [h, w - h]


def _wrap_idxs(arr, w):
    """arr [P, w] int16 (partition-major slot grid) -> [128, w*8] wrapped+replicated."""
    L = P * w
    i = np.arange(L)
    lin = arr[i % P, i // P]                     # list position i = col*128 + p
    wrapped = lin.reshape(w * 8, 16).T           # [16, w*8]
    return np.tile(wrapped, (8, 1)).astype(np.int16)


def _prep_inputs(content_input, W, b):
    """Returns (in_maps, widths) where widths[t][c] is shared across cores."""
    ids = np.asarray(content_input).astype(np.int64).reshape(B, S, K)
    Wf = np.asarray(W, dtype=np.float32)
    bf = np.asarray(b, dtype=np.float32)

    ids = np.where(ids == 0, ZERO_V, ids)
    all_bags = ids.reshape(B * S, K)
    dup_ids, dup_rank = _pick_dup_ids(all_bags)
    tabs = _build_tables(Wf, bf, dup_ids)
    per_core = []
    tiles_per_core = []
    widths = np.zeros((N_TILES, N_CHUNKS), np.int64)
    for c in range(N_CORES):
        bag_ids = ids[c * B_PER_CORE : (c + 1) * B_PER_CORE].reshape(BAGS, K)
        plan = _plan_core(bag_ids, dup_rank)  # bias added on-device
        per_core.append(plan)
        tiles, m = _cluster(plan[1])
        tiles_per_core.append(tiles)
        widths = np.maximum(widths, m)

    bias_rep = np.ascontiguousarray(np.tile(bf[None, :], (P, 1)).astype(np.float32))
    in_maps = []
    for core in range(N_CORES):
        IDX_sorted, cnt = per_core[core]
        tiles = tiles_per_core[core]
        planes = []
        for t in range(N_TILES):
            bags = tiles[t]  # [P] bag ids, -1 = dummy
            real = bags >= 0
            bsafe = np.where(real, bags, 0)
            csum = np.zeros((P,), np.int64)
            for ch in range(N_CHUNKS):
                w = int(widths[t][ch])
                if w == 0:
                    continue
                cn = np.where(real, cnt[bsafe, ch], 0)
                j = np.arange(w)[None, :]
                src = np.take_along_axis(
                    IDX_sorted[bsafe], np.minimum(csum[:, None] + j, K - 1), axis=1
                )
                arr = np.where((j < cn[:, None]) & real[:, None], src, 0).astype(
                    np.int16
                )
                a = 0
                for pw in _pieces(w):
                    planes.append(_wrap_idxs(arr[:, a : a + pw], pw))
                    a += pw
                csum += cn
        ids16 = np.concatenate(planes, axis=1)
        m = {"ids16": np.ascontiguousarray(ids16), "bias": bias_rep}
        for c in range(N_CHUNKS):
            m[f"wt{c}"] = tabs[c]
        in_maps.append(m)
    return in_maps, widths, tiles_per_core


def _build_program(widths, ids_cols):
    import concourse.bass as bass
    import concourse.mybir as mybir
    from concourse import bacc
    from concourse.tile import TileContext

    sw = [int(widths[t].sum()) for t in range(N_TILES)]
    sw_max = max(sw)
    # g tile is sw_max*256B per partition; keep total pool under ~160KB/partition
    g_bufs = max(1, min(8, (120 * 1024) // (sw_max * E * 2)))

    nc = bacc.Bacc("TRN2", target_bir_lowering=False, debug=False,
                   num_devices=N_CORES, num_swdge_queues=4,
                   dynamic_dma_scratch_size=32768)
    ids_d = nc.declare_dram_parameter("ids16", [P, ids_cols], mybir.dt.int16, isOutput=False)
    wt_ds = [
        nc.declare_dram_parameter(
            f"wt{c}", [CHUNK_ROWS[c], E], mybir.dt.bfloat16, isOutput=False
        )
        for c in range(N_CHUNKS)
    ]
    bias_d = nc.declare_dram_parameter("bias", [P, E], mybir.dt.float32, isOutput=False)
    out_d = nc.declare_dram_parameter("out", [BAGS_PAD, E], mybir.dt.float32, isOutput=True)

    with TileContext(nc) as tc:
        with (
            tc.tile_pool(name="ids", bufs=1) as ids_pool,
            tc.tile_pool(name="bias", bufs=1) as bias_pool,
            tc.tile_pool(name="g", bufs=g_bufs) as g_pool,
            tc.tile_pool(name="o", bufs=3) as o_pool,
        ):
            bias_sb = bias_pool.tile([P, E], mybir.dt.float32)
            nc.sync.dma_start(out=bias_sb[:], in_=bias_d[:])
            # split the ids load so tile 0's gathers only wait for its slice
            t0_cols = int(widths[0].sum()) * 8
            ids_sb = ids_pool.tile([P, t0_cols], mybir.dt.int16)
            nc.sync.dma_start(out=ids_sb[:], in_=ids_d[:, :t0_cols])
            ids_sb2 = ids_pool.tile([P, ids_cols - t0_cols], mybir.dt.int16)
            nc.sync.dma_start(out=ids_sb2[:], in_=ids_d[:, t0_cols:])
            off = 0  # free-dim offset into ids16, in idx elements
            qi = 0   # emitted-gather counter; queue = qi % 4 keeps each of
            # Tile's 8 round-robin DMASW lanes locked to a single SWDGE queue
            for t in range(N_TILES):
                # two half-tile buffers: chunks 0-1 -> gA, 2-3 -> gB, so the
                # first half-reduce overlaps the second half's gathers
                wA = int(widths[t][0] + widths[t][1])
                wB = int(widths[t][2] + widths[t][3])
                gA = g_pool.tile([P, max(wA, 1) * E], mybir.dt.bfloat16)
                gB = g_pool.tile([P, max(wB, 1) * E], mybir.dt.bfloat16)
                col = 0
                for ch in range(N_CHUNKS):
                    w = int(widths[t][ch])
                    if w == 0:
                        continue
                    if ch == 2:
                        col = 0
                    g = gA if ch < 2 else gB
                    for pw in _pieces(w):
                        n = pw * P
                        dst = g[:, col * E : (col + pw) * E].rearrange(
                            "p (j e) -> p j e", j=pw, e=E
                        )
                        if off < t0_cols:
                            idx_ap = ids_sb[:, off : off + pw * 8]
                        else:
                            idx_ap = ids_sb2[:, off - t0_cols : off - t0_cols + pw * 8]
                        nc.gpsimd.dma_gather(
                            dst,
                            wt_ds[ch][:],
                            idx_ap,
                            n,
                            n,
                            E,
                            single_packet=False,
                            queue_num=qi % 4,
                        )
                        qi += 1
                        col += pw
                        off += pw * 8
                oA = o_pool.tile([P, E], mybir.dt.float32)
                nc.vector.tensor_reduce(
                    out=oA[:],
                    in_=gA[:, : wA * E].rearrange("p (j e) -> p e j", j=wA, e=E),
                    axis=mybir.AxisListType.X,
                    op=mybir.AluOpType.add,
                )
                o = o_pool.tile([P, E], mybir.dt.float32)
                nc.vector.tensor_reduce(
                    out=o[:],
                    in_=gB[:, : wB * E].rearrange("p (j e) -> p e j", j=wB, e=E),
                    axis=mybir.AxisListType.X,
                    op=mybir.AluOpType.add,
                )
                nc.vector.tensor_add(out=o[:], in0=o[:], in1=oA[:])
                nc.vector.tensor_add(out=o[:], in0=o[:], in1=bias_sb[:])
                nc.sync.dma_start(out=out_d[t * P : (t + 1) * P, :], in_=o[:])
    nc.compile()
    return nc


def kernel(content_input, W, b):
    global LAST_RESULTS
    in_maps, widths, tiles_per_core = _prep_inputs(content_input, W, b)
    ids_cols = in_maps[0]["ids16"].shape[1]
    nc = _build_program(widths, ids_cols)

    from concourse.bass_utils import run_bass_kernel_spmd

    res = run_bass_kernel_spmd(nc, in_maps, list(range(N_CORES)), trace=TRACE)
    LAST_RESULTS = {
        "exec_time_ns": res.exec_time_ns,
        "mean_exec_time_ns": res.mean_exec_time_ns,
        "instructions_and_trace": res.instructions_and_trace,
        "profile_json": res.profile_json,
        "widths": widths,
    }

    out = np.empty((B, S, E), np.float32)
    for c in range(N_CORES):
        rows = res.results[c]["out"]  # [BAGS_PAD, E], permuted bag order
        flat = np.empty((BAGS, E), np.float32)
        tiles = tiles_per_core[c].reshape(BAGS_PAD)
        real = tiles >= 0
        flat[tiles[real]] = rows[real]
        out[c * B_PER_CORE : (c + 1) * B_PER_CORE] = flat.reshape(B_PER_CORE, S, E)
    return out

